# revision 51
# baseline (speedup 1.0000x reference)
"""Bass/Tile TRN2 kernel for nn_MultiHeadAttention_4329327034628.

Multi-head self-attention with additive position bias + causal mask
(T5-style), B=2, Q=2048, D=768, H=12, DKV=64, fp32.

Sharding over 8 NeuronCores: core k -> (batch b = k//4, head-group
g = k%4 of 3 heads).  Each core computes its heads' attention and a
partial output projection (attn @ Wo_slice); the host sums the 4
partials per batch (the post-Wo all-reduce done at gather time).

Device-side layout strategy (no on-chip transposes needed):
  - host ships X^T (D on partitions) -> QKV projections contract D.
  - Q^T, K^T kept as [dkv, q]; scores computed transposed:
      scores^T [k, q] = lhsT(K^T slice).T @ rhs(Q^T)   (contract dkv)
  - position_bias is pre-transposed on host to [k, q] tiles, the causal
    NEG added, and *exponentiated* (expb = exp(biasT + causal), fp16):
    exp(s + b) = exp(s) * exp(b), so the device does ACT exp(s) followed
    by a cheap fp16 2x-mode DVE multiply -- no fp32 PSUM add needed.
    Masked entries have expb == 0 exactly -> probs match the reference.
  - attention_mask indexes k = partitions -> fused into the Exp
    activation as a per-partition bias.
  - softmax without max-subtraction (scores bounded by ~ +-10).
  - row-sum of exp fused into the AV matmul via a ones column:
      lhsT = [V_h | 1] [128k, 65] -> out rows 0..63 = out^T, row 64 = sum.
  - normalization: recip(sum) broadcast via ones-matmul, DVE multiply.
  - Wo: lhsT = attnT_h [64, 128q], rhs = Wo slice [64, 384] -> natural
    [q, D] partial output, DMA'd out.
"""

import numpy as np

B, Q, D, H, DKV = 2, 2048, 768, 12, 64
HPC = 3              # heads per core
NCORES = 8
NEG = -30000.0       # causal mask addend; exp(x + NEG) == 0.0 in fp32
QC = 512             # q chunk (moving dim)
KT = 128             # k tile (partition dim)
NQC = Q // QC        # 4
NKT = Q // KT        # 16
DC = D // 128        # 6 contraction chunks

_prog_cache = {}


def _build_program():
    import concourse.bass as bass
    import concourse.tile as tile
    from concourse import bacc, mybir
    from concourse.bass import ts

    F32 = mybir.dt.float32
    F16 = mybir.dt.float16
    EXP = mybir.ActivationFunctionType.Exp

    nc = bacc.Bacc("TRN2", target_bir_lowering=False, debug=False)

    xT = nc.dram_tensor("xT", [128, DC, Q], F16, kind="ExternalInput").ap()
    # cols 0:128 = Wq' heads {0,1}; 128:192 = Wq' head 2;
    # 192:320 = Wk heads {0,1}; 320:384 = Wk head 2
    wqk = nc.dram_tensor("wqk", [128, DC, 2 * HPC * DKV], F16, kind="ExternalInput").ap()
    wv = nc.dram_tensor("wv", [128, DC, HPC * DKV], F16, kind="ExternalInput").ap()
    wo = nc.dram_tensor("wo", [HPC * DKV, D], F16, kind="ExternalInput").ap()
    expb = nc.dram_tensor("expb", [HPC, NKT, 128, Q], F16, kind="ExternalInput").ap()
    out = nc.dram_tensor("out", [NKT, 128, D], F32, kind="ExternalOutput").ap()

    with tile.TileContext(nc) as tc:
        with (
            nc.allow_low_precision(reason="fp16 matmul operands; fp32 psum accum"),
            tc.tile_pool(name="const", bufs=1) as const,
            tc.tile_pool(name="ps", bufs=2, space="PSUM") as ps,
            tc.tile_pool(name="projps", bufs=1, space="PSUM") as projps,
            tc.tile_pool(name="psO", bufs=2, space="PSUM") as psO,
            tc.tile_pool(name="psF", bufs=1, space="PSUM") as psF,
            tc.tile_pool(name="biasp", bufs=5) as biasp,
            tc.tile_pool(name="expsp", bufs=6) as expsp,
            tc.tile_pool(name="expp", bufs=6) as expp,
            tc.tile_pool(name="small", bufs=3) as small,
            tc.tile_pool(name="outp", bufs=3) as outp,
        ):
            # ---- stage A: load everything ----
            from concourse.tile import add_dep_helper
            wqk_sb = const.tile([128, DC, 2 * HPC * DKV], F16, tag="wqk")
            nc.sync.dma_start(wqk_sb[:], wqk[:])
            wv_sb = const.tile([128, DC, HPC * DKV], F16, tag="wv")
            nc.sync.dma_start(wv_sb[:], wv[:])
            # Wo stacked: [0:128] = heads {0,1} vertically, wo2 = head 2
            wo01_sb = const.tile([2 * DKV, D], F16, tag="wo01")
            nc.sync.dma_start(wo01_sb[:], wo[0:2 * DKV, :])
            wo2_sb = const.tile([DKV, D], F16, tag="wo2")
            nc.sync.dma_start(wo2_sb[:], wo[2 * DKV:, :])
            xT_sb = const.tile([128, DC, Q], F16, tag="xT")
            xT_dmas = [
                nc.sync.dma_start(xT_sb[:, c, :], xT[:, c, :])
                for c in range(DC)
            ]
            ones1 = const.tile([1, DKV], F16, tag="ones1")
            nc.gpsimd.memset(ones1[:], 1.0)

            # ---- stage B: projections ----
            # Q^T / K^T storage. Heads 0,1 paired in [128, Q] tiles (head 0 =
            # rows 0:64, head 1 = rows 64:128, so scores-matmul operands share
            # a base partition); head 2 in separate [64, Q] tiles (base 0).
            qT01 = const.tile([128, Q], F16, tag="qT01")
            kT01 = const.tile([128, Q], F16, tag="kT01")
            qT2 = const.tile([DKV, Q], F16, tag="qT2")
            kT2 = const.tile([DKV, Q], F16, tag="kT2")
            # (lhsT weight slice, dest tile) per projection matmul group
            proj_groups = [
                ((0, 128), qT01), ((128, 192), qT2),
                ((192, 320), kT01), ((320, 384), kT2),
            ]
            def make_qk_unit(j, g):
                (w0, w1), dst = proj_groups[g]

                def qk_unit():
                    p = projps.tile([w1 - w0, QC], F32, tag="pp", name="p")
                    for c in range(DC):
                        nc.tensor.matmul(
                            p[:], wqk_sb[:, c, w0:w1], xT_sb[:, c, ts(j, QC)],
                            start=(c == 0), stop=(c == DC - 1),
                        )
                    nc.scalar.copy(dst[:, ts(j, QC)], p[:])
                return qk_unit

            def qk_slices(h, t, j):
                """(lhsT k-slice, rhs q-slice) for head h, k-tile t, q-chunk j."""
                if h == 0:
                    return kT01[0:DKV, ts(t, KT)], qT01[0:DKV, ts(j, QC)]
                if h == 1:
                    return kT01[DKV:128, ts(t, KT)], qT01[DKV:128, ts(j, QC)]
                return kT2[:, ts(t, KT)], qT2[:, ts(j, QC)]

            # V (natural [k, d]) with a ones column per head: [128, NKT, 3, 65]
            # per (tile t, head h): v1[:, t, h, 0:64] = V_h, v1[:, t, h, 64] = 1
            v1 = const.tile([128, NKT, HPC, DKV + 1], F16, tag="v1")
            nc.gpsimd.memset(v1[:], 1.0)

            def make_v_unit(t):
                def v_unit():
                    pv = projps.tile([128, HPC * DKV], F32, tag="pp", name="pv")
                    for c in range(DC):
                        nc.tensor.matmul(
                            pv[:], xT_sb[:, c, ts(t, KT)], wv_sb[:, c, :],
                            start=(c == 0), stop=(c == DC - 1),
                        )
                    # single strided copy: [128, 3, 64] <- [128, (3 64)]
                    nc.vector.tensor_copy(
                        v1[:, t, :, 0:DKV],
                        pv[:].rearrange("p (h d) -> p h d", h=HPC),
                    )
                return v_unit

            # ---- stage C: attention (scores^T layout), stage D: Wo ----
            attnT01 = const.tile([2 * DKV, Q], F16, tag="attnT01")
            attnT2 = const.tile([DKV, Q], F16, tag="attnT2")
            def make_norm(po, h, j):
                # normalize: attnT_h[:, jq] = po[0:64] * (1/po[64]) bcast
                def norm():
                    rc = small.tile([1, QC], F16, tag="rc", name="rc")
                    nc.vector.reciprocal(rc[:], po[64:65, :])
                    bc = small.tile([DKV, QC], F16, tag="bc", name="bc")
                    nc.gpsimd.partition_broadcast(bc[:], rc[:])
                    if h == 0:
                        dst = attnT01[0:DKV, ts(j, QC)]
                    elif h == 2:
                        dst = attnT2[:, ts(j, QC)]
                    else:
                        dst = None
                    if dst is not None:
                        nc.vector.tensor_mul(dst, po[0:64, :], bc[:])
                    else:
                        # head 1 lands at partitions 64:128 of the stacked
                        # pair tile; DVE can't shift partitions, so stage at
                        # base 0 and DMA-shift (SBUF->SBUF moves are
                        # partition-agnostic)
                        stg = small.tile([DKV, QC], F16, tag="stg", name="stg")
                        nc.vector.tensor_mul(stg[:], po[0:64, :], bc[:])
                        nc.sync.dma_start(attnT01[DKV:128, ts(j, QC)], stg[:])
                return norm

            def make_wo(i, tail=False):
                # Wo partial for q-tile i (128 q rows)
                def wo_i():
                    ot = outp.tile([128, D], F32, tag="ot", name="ot")
                    for half in range(2):
                        if tail and half == 1:
                            pf = ps.tile([128, 384], F32, tag="ps", name="pf")
                        else:
                            pf = psF.tile([128, 384], F32, tag="pf", name="pf")
                        hs = slice(384 * half, 384 * half + 384)
                        nc.tensor.matmul(
                            pf[:], attnT01[:, ts(i, KT)], wo01_sb[:, hs],
                            start=True, stop=False,
                        )
                        nc.tensor.matmul(
                            pf[:], attnT2[:, ts(i, KT)], wo2_sb[:, hs],
                            start=False, stop=True,
                        )
                        if half == 0:
                            nc.vector.tensor_copy(ot[:, 0:384], pf[:])
                        else:
                            nc.scalar.copy(ot[:, 384:768], pf[:])
                    nc.sync.dma_start(out[i], ot[:])
                return wo_i

            # Software pipelining via two drip queues:
            #  - normq: deferred normalization + Wo closures (FIFO keeps
            #    norm(h,j) ahead of wo(j,*) which reads normalized attnT);
            #    popped at (h,j) / tt boundaries so po slots recycle.
            #  - projq: projection units for q-chunk j+1 (Q/K chunk matmuls,
            #    V k-tiles), dripped one per t-step of attention(j) so the
            #    serial projection phase disappears into attention's PE gaps.
            normq = []
            projq = []
            n_bt_dmas = [0]
            # chunk-0 projections must precede attention(0)
            for g in range(len(proj_groups)):
                make_qk_unit(0, g)()
            for t in range(4):
                make_v_unit(t)()

            for j in range(NQC):
                if j + 1 < NQC:
                    for g in range(len(proj_groups)):
                        projq.append(make_qk_unit(j + 1, g))
                    for t in range(4 * (j + 1), 4 * (j + 1) + 4):
                        projq.append(make_v_unit(t))
                # drip projections evenly: 8 units over this j's 12(j+1)
                # t-steps, front-loaded enough to finish before attn(j+1)
                stride = max(1, (6 * (j + 1)) // 9)
                slot = 0
                for h in range(HPC):
                    # free po slots before claiming one (po bufs=2)
                    for _ in range(min(2, len(normq))):
                        normq.pop(0)()
                    po = psO.tile([65, QC], F32, tag="po")
                    nkt = 4 * j + 4  # causal: k-tiles 0..4j+3
                    for tt in range(j + 1):  # batched expb DMA: 4 k-tiles
                        if normq:
                            normq.pop(0)()
                        bt = biasp.tile([128, 4, QC], F16, tag="bt")
                        bt_dma = nc.sync.dma_start(
                            bt[:],
                            expb[h, 4 * tt:4 * tt + 4, :, ts(j, QC)]
                            .rearrange("t p q -> p t q"),
                        )
                        if n_bt_dmas[0] < 4:
                            # don't let early expb prefetch steal HBM
                            # bandwidth from the critical-path xT load
                            add_dep_helper(
                                bt_dma.ins, xT_dmas[-1].ins,
                                reason="expb prefetch after xT",
                            )
                        n_bt_dmas[0] += 1
                        for pr in range(2):  # two k-tile pairs per tt-group
                            pss = ps.tile([128, 2, QC], F32, tag="ps")
                            for half in range(2):
                                t = 4 * tt + 2 * pr + half
                                k_sl, q_sl = qk_slices(h, t, j)
                                nc.tensor.matmul(
                                    pss[:, half, :], k_sl, q_sl,
                                    start=True, stop=True,
                                )
                            es = expsp.tile([128, 2, QC], F16, tag="es")
                            nc.scalar.activation(es[:], pss[:], EXP)
                            et = expp.tile([128, 2, QC], F16, tag="et")
                            nc.vector.tensor_mul(
                                et[:], es[:], bt[:, 2 * pr:2 * pr + 2, :]
                            )
                            for half in range(2):
                                t = 4 * tt + 2 * pr + half
                                nc.tensor.matmul(
                                    po[:], v1[:, t, h, :], et[:, half, :],
                                    start=(t == 0), stop=(t == nkt - 1),
                                )
                            if projq and slot % stride == 0:
                                projq.pop(0)()
                            slot += 1
                    normq.append(make_norm(po, h, j))
                for i in range(4 * j, 4 * j + 4):
                    normq.append(make_wo(i, tail=(j == NQC - 1)))
            for fn in normq + projq:
                fn()

    nc.compile()
    return nc


def get_program():
    if "nc" not in _prog_cache:
        _prog_cache["nc"] = _build_program()
    return _prog_cache["nc"]


def make_in_maps(hidden_states, attention_mask, position_bias, Wq, Wk, Wv, Wo):
    hs = np.ascontiguousarray(np.asarray(hidden_states, dtype=np.float32))
    am = np.asarray(attention_mask, dtype=np.float32)
    pb = np.asarray(position_bias, dtype=np.float32)
    wq = np.asarray(Wq, dtype=np.float32) * np.float32(1.0 / np.sqrt(DKV))
    wk = np.asarray(Wk, dtype=np.float32)
    wv_ = np.asarray(Wv, dtype=np.float32)
    wo_ = np.asarray(Wo, dtype=np.float32)

    # causal addend in [k, q] indexing: NEG where k > q
    kk = np.arange(Q, dtype=np.int64)
    causal_T = np.where(kk[:, None] > kk[None, :], np.float32(NEG), np.float32(0.0))
    causal_T = causal_T.astype(np.float32)

    in_maps = []
    for core in range(NCORES):
        b, g = divmod(core, NCORES // B)
        h0 = g * HPC
        # X^T chunked: [128, DC, Q], [p, c, q] = hs[b, q, 128c+p]
        xT = np.ascontiguousarray(
            hs[b].T.reshape(DC, 128, Q).transpose(1, 0, 2)
        ).astype(np.float16)
        # wqk: [128, DC, 384]: cols 0:192 = Wq' slice, 192:384 = Wk slice
        wq_sl = wq[:, h0 * DKV:(h0 + HPC) * DKV]
        wk_sl = wk[:, h0 * DKV:(h0 + HPC) * DKV]
        wqk = np.concatenate([wq_sl, wk_sl], axis=1)  # (D, 384)
        wqk = np.ascontiguousarray(
            wqk.reshape(DC, 128, 2 * HPC * DKV).transpose(1, 0, 2)
        ).astype(np.float16)
        # wv: [128, DC, HPC*DKV]
        wv_sl = wv_[:, (h0) * DKV:(h0 + HPC) * DKV].reshape(DC, 128, HPC * DKV)
        wv_sl = np.ascontiguousarray(wv_sl.transpose(1, 0, 2)).astype(np.float16)
        # wo: [DKV, HPC, D]: [p, h, n] = Wo[(h0+h)*DKV + p, n]
        wo_sl = np.ascontiguousarray(
            wo_[h0 * DKV:(h0 + HPC) * DKV, :]
        ).astype(np.float16)
        # expb: [HPC, NKT, 128, Q]: exp(biasT + causal + mask_k); masked -> 0
        # (attention_mask indexes k, which is the row dim of the transposed
        # bias, so it folds in as a per-row addend before the exp)
        bT = pb[0, h0:h0 + HPC].transpose(0, 2, 1) + causal_T[None]
        bT += am[b, 0, 0][None, :, None]
        bT = np.exp(bT, out=bT)
        bT = np.ascontiguousarray(bT.reshape(HPC, NKT, 128, Q)).astype(np.float16)
        in_maps.append({
            "xT": xT, "wqk": wqk, "wv": wv_sl, "wo": wo_sl,
            "expb": bT,
        })
    return in_maps


def kernel(hidden_states, attention_mask, position_bias, Wq, Wk, Wv, Wo):
    from concourse.bass_utils import run_bass_kernel_spmd

    nc = get_program()
    in_maps = make_in_maps(
        hidden_states, attention_mask, position_bias, Wq, Wk, Wv, Wo
    )
    res = run_bass_kernel_spmd(nc, in_maps, list(range(NCORES)))
    out = np.zeros((B, Q, D), dtype=np.float32)
    for core in range(NCORES):
        b = core // (NCORES // B)
        out[b] += res.results[core]["out"].reshape(Q, D)
    return out


# revision 55
# speedup vs baseline: 1.0521x; 1.0521x over previous
"""Bass/Tile TRN2 kernel for nn_MultiHeadAttention_4329327034628.

Multi-head self-attention with additive position bias + causal mask
(T5-style), B=2, Q=2048, D=768, H=12, DKV=64, fp32.

Sharding over 8 NeuronCores: core k -> (batch b = k//4, head-group
g = k%4 of 3 heads).  Each core computes its heads' attention and a
partial output projection (attn @ Wo_slice); the host sums the 4
partials per batch (the post-Wo all-reduce done at gather time).

Device-side layout strategy (no on-chip transposes needed):
  - host ships X^T (D on partitions) -> QKV projections contract D.
  - Q^T, K^T kept as [dkv, q]; scores computed transposed:
      scores^T [k, q] = lhsT(K^T slice).T @ rhs(Q^T)   (contract dkv)
  - position_bias is pre-transposed on host to [k, q] tiles, the causal
    NEG added, and *exponentiated* (expb = exp(biasT + causal), fp16):
    exp(s + b) = exp(s) * exp(b), so the device does ACT exp(s) followed
    by a cheap fp16 2x-mode DVE multiply -- no fp32 PSUM add needed.
    Masked entries have expb == 0 exactly -> probs match the reference.
  - attention_mask indexes k = partitions -> fused into the Exp
    activation as a per-partition bias.
  - softmax without max-subtraction (scores bounded by ~ +-10).
  - row-sum of exp fused into the AV matmul via a ones column:
      lhsT = [V_h | 1] [128k, 65] -> out rows 0..63 = out^T, row 64 = sum.
  - normalization: recip(sum) broadcast via ones-matmul, DVE multiply.
  - Wo: lhsT = attnT_h [64, 128q], rhs = Wo slice [64, 384] -> natural
    [q, D] partial output, DMA'd out.
"""

import numpy as np

B, Q, D, H, DKV = 2, 2048, 768, 12, 64
HPC = 3              # heads per core
NCORES = 8
NEG = -30000.0       # causal mask addend; exp(x + NEG) == 0.0 in fp32
QC = 512             # q chunk (moving dim)
KT = 128             # k tile (partition dim)
NQC = Q // QC        # 4
NKT = Q // KT        # 16
DC = D // 128        # 6 contraction chunks

_prog_cache = {}


def _build_program():
    import concourse.bass as bass
    import concourse.tile as tile
    from concourse import bacc, mybir
    from concourse.bass import ts

    F32 = mybir.dt.float32
    F16 = mybir.dt.float16
    EXP = mybir.ActivationFunctionType.Exp

    nc = bacc.Bacc("TRN2", target_bir_lowering=False, debug=False)

    xT = nc.dram_tensor("xT", [128, DC, Q], F16, kind="ExternalInput").ap()
    # cols 0:128 = Wq' heads {0,1}; 128:192 = Wq' head 2;
    # 192:320 = Wk heads {0,1}; 320:384 = Wk head 2
    wqk = nc.dram_tensor("wqk", [128, DC, 2 * HPC * DKV], F16, kind="ExternalInput").ap()
    wv = nc.dram_tensor("wv", [128, DC, HPC * DKV], F16, kind="ExternalInput").ap()
    wo = nc.dram_tensor("wo", [HPC * DKV, D], F16, kind="ExternalInput").ap()
    expb = nc.dram_tensor("expb", [HPC, NKT, 128, Q], F16, kind="ExternalInput").ap()
    out = nc.dram_tensor("out", [NKT, 128, D], F32, kind="ExternalOutput").ap()

    with tile.TileContext(nc) as tc:
        with (
            nc.allow_low_precision(reason="fp16 matmul operands; fp32 psum accum"),
            tc.tile_pool(name="const", bufs=1) as const,
            tc.tile_pool(name="ps", bufs=2, space="PSUM") as ps,
            tc.tile_pool(name="projps", bufs=1, space="PSUM") as projps,
            tc.tile_pool(name="psO", bufs=2, space="PSUM") as psO,
            tc.tile_pool(name="psF", bufs=1, space="PSUM") as psF,
            tc.tile_pool(name="biasp", bufs=5) as biasp,
            tc.tile_pool(name="expsp", bufs=6) as expsp,
            tc.tile_pool(name="expp", bufs=6) as expp,
            tc.tile_pool(name="small", bufs=3) as small,
            tc.tile_pool(name="outp", bufs=3) as outp,
        ):
            # ---- stage A: load everything ----
            from concourse.tile import add_dep_helper
            wqk_sb = const.tile([128, DC, 2 * HPC * DKV], F16, tag="wqk")
            nc.sync.dma_start(wqk_sb[:], wqk[:])
            wv_sb = const.tile([128, DC, HPC * DKV], F16, tag="wv")
            nc.sync.dma_start(wv_sb[:], wv[:])
            # Wo stacked: [0:128] = heads {0,1} vertically, wo2 = head 2
            wo01_sb = const.tile([2 * DKV, D], F16, tag="wo01")
            nc.sync.dma_start(wo01_sb[:], wo[0:2 * DKV, :])
            wo2_sb = const.tile([DKV, D], F16, tag="wo2")
            nc.sync.dma_start(wo2_sb[:], wo[2 * DKV:, :])
            xT_sb = const.tile([128, DC, Q], F16, tag="xT")
            xT_dmas = [
                nc.sync.dma_start(
                    xT_sb[:, c, ts(hf, Q // 2)], xT[:, c, ts(hf, Q // 2)]
                )
                for c in range(DC)
                for hf in range(2)
            ]
            ones1 = const.tile([1, DKV], F16, tag="ones1")
            nc.gpsimd.memset(ones1[:], 1.0)

            # ---- stage B: projections ----
            # Q^T / K^T storage. Heads 0,1 paired in [128, Q] tiles (head 0 =
            # rows 0:64, head 1 = rows 64:128, so scores-matmul operands share
            # a base partition); head 2 in separate [64, Q] tiles (base 0).
            qT01 = const.tile([128, Q], F16, tag="qT01")
            kT01 = const.tile([128, Q], F16, tag="kT01")
            qT2 = const.tile([DKV, Q], F16, tag="qT2")
            kT2 = const.tile([DKV, Q], F16, tag="kT2")
            # (lhsT weight slice, dest tile) per projection matmul group
            proj_groups = [
                ((0, 128), qT01), ((128, 192), qT2),
                ((192, 320), kT01), ((320, 384), kT2),
            ]
            def make_qk_unit(j, g, pool_tag=None):
                (w0, w1), dst = proj_groups[g]

                def qk_unit():
                    pool, tag = pool_tag or (projps, "pp")
                    p = pool.tile([w1 - w0, QC], F32, tag=tag, name="p")
                    for c in range(DC):
                        nc.tensor.matmul(
                            p[:], wqk_sb[:, c, w0:w1], xT_sb[:, c, ts(j, QC)],
                            start=(c == 0), stop=(c == DC - 1),
                        )
                    nc.scalar.copy(dst[:, ts(j, QC)], p[:])
                return qk_unit

            def qk_slices(h, t, j):
                """(lhsT k-slice, rhs q-slice) for head h, k-tile t, q-chunk j."""
                if h == 0:
                    return kT01[0:DKV, ts(t, KT)], qT01[0:DKV, ts(j, QC)]
                if h == 1:
                    return kT01[DKV:128, ts(t, KT)], qT01[DKV:128, ts(j, QC)]
                return kT2[:, ts(t, KT)], qT2[:, ts(j, QC)]

            # V (natural [k, d]) with a ones column per head: [128, NKT, 3, 65]
            # per (tile t, head h): v1[:, t, h, 0:64] = V_h, v1[:, t, h, 64] = 1
            v1 = const.tile([128, NKT, HPC, DKV + 1], F16, tag="v1")
            nc.gpsimd.memset(v1[:], 1.0)

            def make_v_unit(t, pool_tag=None):
                def v_unit():
                    pool, tag = pool_tag or (projps, "pp")
                    pv = pool.tile([128, HPC * DKV], F32, tag=tag, name="pv")
                    for c in range(DC):
                        nc.tensor.matmul(
                            pv[:], xT_sb[:, c, ts(t, KT)], wv_sb[:, c, :],
                            start=(c == 0), stop=(c == DC - 1),
                        )
                    # single strided copy: [128, 3, 64] <- [128, (3 64)]
                    nc.vector.tensor_copy(
                        v1[:, t, :, 0:DKV],
                        pv[:].rearrange("p (h d) -> p h d", h=HPC),
                    )
                return v_unit

            # ---- stage C: attention (scores^T layout), stage D: Wo ----
            attnT01 = const.tile([2 * DKV, Q], F16, tag="attnT01")
            attnT2 = const.tile([DKV, Q], F16, tag="attnT2")
            def make_norm(po, h, j):
                # normalize: attnT_h[:, jq] = po[0:64] * (1/po[64]) bcast
                def norm():
                    rc = small.tile([1, QC], F16, tag="rc", name="rc")
                    nc.vector.reciprocal(rc[:], po[64:65, :])
                    bc = small.tile([DKV, QC], F16, tag="bc", name="bc")
                    nc.gpsimd.partition_broadcast(bc[:], rc[:])
                    if h == 0:
                        dst = attnT01[0:DKV, ts(j, QC)]
                    elif h == 2:
                        dst = attnT2[:, ts(j, QC)]
                    else:
                        dst = None
                    if dst is not None:
                        nc.vector.tensor_mul(dst, po[0:64, :], bc[:])
                    else:
                        # head 1 lands at partitions 64:128 of the stacked
                        # pair tile; DVE can't shift partitions, so stage at
                        # base 0 and DMA-shift (SBUF->SBUF moves are
                        # partition-agnostic)
                        stg = small.tile([DKV, QC], F16, tag="stg", name="stg")
                        nc.vector.tensor_mul(stg[:], po[0:64, :], bc[:])
                        nc.sync.dma_start(attnT01[DKV:128, ts(j, QC)], stg[:])
                return norm

            def make_wo(i, tail=False):
                # Wo partial for q-tile i (128 q rows)
                def wo_i():
                    ot = outp.tile([128, D], F32, tag="ot", name="ot")
                    for half in range(2):
                        if tail and half == 1:
                            pf = ps.tile([128, 384], F32, tag="ps", name="pf")
                        else:
                            pf = psF.tile([128, 384], F32, tag="pf", name="pf")
                        hs = slice(384 * half, 384 * half + 384)
                        nc.tensor.matmul(
                            pf[:], attnT01[:, ts(i, KT)], wo01_sb[:, hs],
                            start=True, stop=False,
                        )
                        nc.tensor.matmul(
                            pf[:], attnT2[:, ts(i, KT)], wo2_sb[:, hs],
                            start=False, stop=True,
                        )
                        if half == 0:
                            nc.vector.tensor_copy(ot[:, 0:384], pf[:])
                        else:
                            nc.scalar.copy(ot[:, 384:768], pf[:])
                    nc.sync.dma_start(out[i], ot[:])
                return wo_i

            # Software pipelining via two drip queues:
            #  - normq: deferred normalization + Wo closures (FIFO keeps
            #    norm(h,j) ahead of wo(j,*) which reads normalized attnT);
            #    popped at (h,j) / tt boundaries so po slots recycle.
            #  - projq: projection units for q-chunk j+1 (Q/K chunk matmuls,
            #    V k-tiles), dripped one per t-step of attention(j) so the
            #    serial projection phase disappears into attention's PE gaps.
            normq = []
            projq = []
            n_bt_dmas = [0]
            # chunk-0 projections must precede attention(0); attention is not
            # running yet, so spread them over the idle pair-pool psum slots
            # to pipeline instead of serializing on the single "pp" slot
            startup_slots = [(projps, "pp"), (ps, "ps"), (psO, "po")]
            for g in range(len(proj_groups)):
                make_qk_unit(0, g, startup_slots[g % 3])()
            for t in range(4):
                make_v_unit(t, startup_slots[t % 3])()

            for j in range(NQC):
                if j + 1 < NQC:
                    for g in range(len(proj_groups)):
                        projq.append(make_qk_unit(j + 1, g))
                    for t in range(4 * (j + 1), 4 * (j + 1) + 4):
                        projq.append(make_v_unit(t))
                # drip projections evenly: 8 units over this j's 12(j+1)
                # t-steps, front-loaded enough to finish before attn(j+1)
                stride = max(1, (6 * (j + 1)) // 9)
                slot = 0
                for h in range(HPC):
                    # free po slots before claiming one (po bufs=2)
                    for _ in range(min(2, len(normq))):
                        normq.pop(0)()
                    po = psO.tile([65, QC], F32, tag="po")
                    nkt = 4 * j + 4  # causal: k-tiles 0..4j+3
                    for tt in range(j + 1):  # batched expb DMA: 4 k-tiles
                        if normq:
                            normq.pop(0)()
                        bt = biasp.tile([128, 4, QC], F16, tag="bt")
                        bt_dma = nc.sync.dma_start(
                            bt[:],
                            expb[h, 4 * tt:4 * tt + 4, :, ts(j, QC)]
                            .rearrange("t p q -> p t q"),
                        )
                        if n_bt_dmas[0] < 4:
                            # don't let early expb prefetch steal HBM
                            # bandwidth from the critical-path xT load
                            add_dep_helper(
                                bt_dma.ins, xT_dmas[-1].ins,
                                reason="expb prefetch after xT",
                            )
                        n_bt_dmas[0] += 1
                        for pr in range(2):  # two k-tile pairs per tt-group
                            pss = ps.tile([128, 2, QC], F32, tag="ps")
                            for half in range(2):
                                t = 4 * tt + 2 * pr + half
                                k_sl, q_sl = qk_slices(h, t, j)
                                nc.tensor.matmul(
                                    pss[:, half, :], k_sl, q_sl,
                                    start=True, stop=True,
                                )
                            es = expsp.tile([128, 2, QC], F16, tag="es")
                            nc.scalar.activation(es[:], pss[:], EXP)
                            et = expp.tile([128, 2, QC], F16, tag="et")
                            nc.vector.tensor_mul(
                                et[:], es[:], bt[:, 2 * pr:2 * pr + 2, :]
                            )
                            for half in range(2):
                                t = 4 * tt + 2 * pr + half
                                nc.tensor.matmul(
                                    po[:], v1[:, t, h, :], et[:, half, :],
                                    start=(t == 0), stop=(t == nkt - 1),
                                )
                            if projq and slot % stride == 0:
                                projq.pop(0)()
                            slot += 1
                    normq.append(make_norm(po, h, j))
                for i in range(4 * j, 4 * j + 4):
                    normq.append(make_wo(i, tail=(j == NQC - 1)))
            for fn in normq + projq:
                fn()

    nc.compile()
    return nc


def get_program():
    if "nc" not in _prog_cache:
        _prog_cache["nc"] = _build_program()
    return _prog_cache["nc"]


def make_in_maps(hidden_states, attention_mask, position_bias, Wq, Wk, Wv, Wo):
    hs = np.ascontiguousarray(np.asarray(hidden_states, dtype=np.float32))
    am = np.asarray(attention_mask, dtype=np.float32)
    pb = np.asarray(position_bias, dtype=np.float32)
    wq = np.asarray(Wq, dtype=np.float32) * np.float32(1.0 / np.sqrt(DKV))
    wk = np.asarray(Wk, dtype=np.float32)
    wv_ = np.asarray(Wv, dtype=np.float32)
    wo_ = np.asarray(Wo, dtype=np.float32)

    # causal addend in [k, q] indexing: NEG where k > q
    kk = np.arange(Q, dtype=np.int64)
    causal_T = np.where(kk[:, None] > kk[None, :], np.float32(NEG), np.float32(0.0))
    causal_T = causal_T.astype(np.float32)

    in_maps = []
    for core in range(NCORES):
        b, g = divmod(core, NCORES // B)
        h0 = g * HPC
        # X^T chunked: [128, DC, Q], [p, c, q] = hs[b, q, 128c+p]
        xT = np.ascontiguousarray(
            hs[b].T.reshape(DC, 128, Q).transpose(1, 0, 2)
        ).astype(np.float16)
        # wqk: [128, DC, 384]: cols 0:192 = Wq' slice, 192:384 = Wk slice
        wq_sl = wq[:, h0 * DKV:(h0 + HPC) * DKV]
        wk_sl = wk[:, h0 * DKV:(h0 + HPC) * DKV]
        wqk = np.concatenate([wq_sl, wk_sl], axis=1)  # (D, 384)
        wqk = np.ascontiguousarray(
            wqk.reshape(DC, 128, 2 * HPC * DKV).transpose(1, 0, 2)
        ).astype(np.float16)
        # wv: [128, DC, HPC*DKV]
        wv_sl = wv_[:, (h0) * DKV:(h0 + HPC) * DKV].reshape(DC, 128, HPC * DKV)
        wv_sl = np.ascontiguousarray(wv_sl.transpose(1, 0, 2)).astype(np.float16)
        # wo: [DKV, HPC, D]: [p, h, n] = Wo[(h0+h)*DKV + p, n]
        wo_sl = np.ascontiguousarray(
            wo_[h0 * DKV:(h0 + HPC) * DKV, :]
        ).astype(np.float16)
        # expb: [HPC, NKT, 128, Q]: exp(biasT + causal + mask_k); masked -> 0
        # (attention_mask indexes k, which is the row dim of the transposed
        # bias, so it folds in as a per-row addend before the exp)
        bT = pb[0, h0:h0 + HPC].transpose(0, 2, 1) + causal_T[None]
        bT += am[b, 0, 0][None, :, None]
        bT = np.exp(bT, out=bT)
        bT = np.ascontiguousarray(bT.reshape(HPC, NKT, 128, Q)).astype(np.float16)
        in_maps.append({
            "xT": xT, "wqk": wqk, "wv": wv_sl, "wo": wo_sl,
            "expb": bT,
        })
    return in_maps


def kernel(hidden_states, attention_mask, position_bias, Wq, Wk, Wv, Wo):
    from concourse.bass_utils import run_bass_kernel_spmd

    nc = get_program()
    in_maps = make_in_maps(
        hidden_states, attention_mask, position_bias, Wq, Wk, Wv, Wo
    )
    res = run_bass_kernel_spmd(nc, in_maps, list(range(NCORES)))
    out = np.zeros((B, Q, D), dtype=np.float32)
    for core in range(NCORES):
        b = core // (NCORES // B)
        out[b] += res.results[core]["out"].reshape(Q, D)
    return out


# revision 56
# speedup vs baseline: 1.0765x; 1.0232x over previous
"""Bass/Tile TRN2 kernel for nn_MultiHeadAttention_4329327034628.

Multi-head self-attention with additive position bias + causal mask
(T5-style), B=2, Q=2048, D=768, H=12, DKV=64, fp32.

Sharding over 8 NeuronCores: core k -> (batch b = k//4, head-group
g = k%4 of 3 heads).  Each core computes its heads' attention and a
partial output projection (attn @ Wo_slice); the host sums the 4
partials per batch (the post-Wo all-reduce done at gather time).

Device-side layout strategy (no on-chip transposes needed):
  - host ships X^T (D on partitions) -> QKV projections contract D.
  - Q^T, K^T kept as [dkv, q]; scores computed transposed:
      scores^T [k, q] = lhsT(K^T slice).T @ rhs(Q^T)   (contract dkv)
  - position_bias is pre-transposed on host to [k, q] tiles, the causal
    NEG added, and *exponentiated* (expb = exp(biasT + causal), fp16):
    exp(s + b) = exp(s) * exp(b), so the device does ACT exp(s) followed
    by a cheap fp16 2x-mode DVE multiply -- no fp32 PSUM add needed.
    Masked entries have expb == 0 exactly -> probs match the reference.
  - attention_mask indexes k = partitions -> fused into the Exp
    activation as a per-partition bias.
  - softmax without max-subtraction (scores bounded by ~ +-10).
  - row-sum of exp fused into the AV matmul via a ones column:
      lhsT = [V_h | 1] [128k, 65] -> out rows 0..63 = out^T, row 64 = sum.
  - normalization: recip(sum) broadcast via ones-matmul, DVE multiply.
  - Wo: lhsT = attnT_h [64, 128q], rhs = Wo slice [64, 384] -> natural
    [q, D] partial output, DMA'd out.
"""

import numpy as np

B, Q, D, H, DKV = 2, 2048, 768, 12, 64
HPC = 3              # heads per core
NCORES = 8
NEG = -30000.0       # causal mask addend; exp(x + NEG) == 0.0 in fp32
QC = 512             # q chunk (moving dim)
KT = 128             # k tile (partition dim)
NQC = Q // QC        # 4
NKT = Q // KT        # 16
DC = D // 128        # 6 contraction chunks

_prog_cache = {}


def _build_program():
    import concourse.bass as bass
    import concourse.tile as tile
    from concourse import bacc, mybir
    from concourse.bass import ts

    F32 = mybir.dt.float32
    F16 = mybir.dt.float16
    EXP = mybir.ActivationFunctionType.Exp

    nc = bacc.Bacc("TRN2", target_bir_lowering=False, debug=False)

    xT = nc.dram_tensor("xT", [128, DC, Q], F16, kind="ExternalInput").ap()
    # cols 0:128 = Wq' heads {0,1}; 128:256 = Wk heads {0,1};
    # 256:320 = Wq' head 2; 320:384 = Wk head 2
    wqk = nc.dram_tensor("wqk", [128, DC, 2 * HPC * DKV], F16, kind="ExternalInput").ap()
    wv = nc.dram_tensor("wv", [128, DC, HPC * DKV], F16, kind="ExternalInput").ap()
    wo = nc.dram_tensor("wo", [HPC * DKV, D], F16, kind="ExternalInput").ap()
    expb = nc.dram_tensor("expb", [HPC, NKT, 128, Q], F16, kind="ExternalInput").ap()
    out = nc.dram_tensor("out", [NKT, 128, D], F16, kind="ExternalOutput").ap()

    with tile.TileContext(nc) as tc:
        with (
            nc.allow_low_precision(reason="fp16 matmul operands; fp32 psum accum"),
            tc.tile_pool(name="const", bufs=1) as const,
            tc.tile_pool(name="ps", bufs=2, space="PSUM") as ps,
            tc.tile_pool(name="projps", bufs=1, space="PSUM") as projps,
            tc.tile_pool(name="psO", bufs=2, space="PSUM") as psO,
            tc.tile_pool(name="psF", bufs=1, space="PSUM") as psF,
            tc.tile_pool(name="biasp", bufs=5) as biasp,
            tc.tile_pool(name="expsp", bufs=6) as expsp,
            tc.tile_pool(name="expp", bufs=6) as expp,
            tc.tile_pool(name="small", bufs=3) as small,
            tc.tile_pool(name="outp", bufs=3) as outp,
        ):
            # ---- stage A: load everything ----
            from concourse.tile import add_dep_helper
            wqk_sb = const.tile([128, DC, 2 * HPC * DKV], F16, tag="wqk")
            for c in range(DC):
                nc.sync.dma_start(wqk_sb[:, c, :], wqk[:, c, :])
            wv_sb = const.tile([128, DC, HPC * DKV], F16, tag="wv")
            nc.sync.dma_start(wv_sb[:], wv[:])
            # Wo stacked: [0:128] = heads {0,1} vertically, wo2 = head 2
            wo01_sb = const.tile([2 * DKV, D], F16, tag="wo01")
            nc.sync.dma_start(wo01_sb[:], wo[0:2 * DKV, :])
            wo2_sb = const.tile([DKV, D], F16, tag="wo2")
            nc.sync.dma_start(wo2_sb[:], wo[2 * DKV:, :])
            xT_sb = const.tile([128, DC, Q], F16, tag="xT")
            xT_dmas = [
                nc.sync.dma_start(
                    xT_sb[:, c, ts(hf, Q // 2)], xT[:, c, ts(hf, Q // 2)]
                )
                for c in range(DC)
                for hf in range(2)
            ]
            ones1 = const.tile([1, DKV], F16, tag="ones1")
            nc.gpsimd.memset(ones1[:], 1.0)

            # ---- stage B: projections ----
            # Q^T / K^T storage. Heads 0,1 paired in [128, Q] tiles (head 0 =
            # rows 0:64, head 1 = rows 64:128, so scores-matmul operands share
            # a base partition); head 2 in separate [64, Q] tiles (base 0).
            qT01 = const.tile([128, Q], F16, tag="qT01")
            kT01 = const.tile([128, Q], F16, tag="kT01")
            # head 2: one M=128 group -> qkT2 rows 0:64 = Q^T, 64:128 = K^T;
            # K^T is then DMA-shifted down to kT2b rows 0:64 so the scores
            # matmul operands share base partition 0
            qkT2 = const.tile([128, Q], F16, tag="qkT2")
            kT2b = const.tile([DKV, Q], F16, tag="kT2b")
            # (lhsT weight slice, dest tile) per projection matmul group
            proj_groups = [
                ((0, 128), qT01), ((128, 256), kT01), ((256, 384), qkT2),
            ]
            def make_qk_unit(j, g, pool_tag=None):
                (w0, w1), dst = proj_groups[g]

                def qk_unit():
                    pool, tag = pool_tag or (projps, "pp")
                    p = pool.tile([w1 - w0, QC], F32, tag=tag, name="p")
                    for c in range(DC):
                        nc.tensor.matmul(
                            p[:], wqk_sb[:, c, w0:w1], xT_sb[:, c, ts(j, QC)],
                            start=(c == 0), stop=(c == DC - 1),
                        )
                    nc.scalar.copy(dst[:, ts(j, QC)], p[:])
                    if dst is qkT2:
                        nc.sync.dma_start(
                            kT2b[:, ts(j, QC)], qkT2[DKV:128, ts(j, QC)]
                        )
                return qk_unit

            def qk_slices(h, t, j):
                """(lhsT k-slice, rhs q-slice) for head h, k-tile t, q-chunk j."""
                if h == 0:
                    return kT01[0:DKV, ts(t, KT)], qT01[0:DKV, ts(j, QC)]
                if h == 1:
                    return kT01[DKV:128, ts(t, KT)], qT01[DKV:128, ts(j, QC)]
                return kT2b[:, ts(t, KT)], qkT2[0:DKV, ts(j, QC)]

            # V (natural [k, d]) with a ones column per head: [128, NKT, 3, 65]
            # per (tile t, head h): v1[:, t, h, 0:64] = V_h, v1[:, t, h, 64] = 1
            v1 = const.tile([128, NKT, HPC, DKV + 1], F16, tag="v1")
            nc.gpsimd.memset(v1[:], 1.0)

            def make_v_unit(t, pool_tag=None):
                def v_unit():
                    pool, tag = pool_tag or (projps, "pp")
                    pv = pool.tile([128, HPC * DKV], F32, tag=tag, name="pv")
                    for c in range(DC):
                        nc.tensor.matmul(
                            pv[:], xT_sb[:, c, ts(t, KT)], wv_sb[:, c, :],
                            start=(c == 0), stop=(c == DC - 1),
                        )
                    # single strided copy: [128, 3, 64] <- [128, (3 64)]
                    nc.vector.tensor_copy(
                        v1[:, t, :, 0:DKV],
                        pv[:].rearrange("p (h d) -> p h d", h=HPC),
                    )
                return v_unit

            # ---- stage C: attention (scores^T layout), stage D: Wo ----
            attnT01 = const.tile([2 * DKV, Q], F16, tag="attnT01")
            attnT2 = const.tile([DKV, Q], F16, tag="attnT2")
            def make_norm(po, h, j):
                # normalize: attnT_h[:, jq] = po[0:64] * (1/po[64]) bcast
                def norm():
                    rc = small.tile([1, QC], F16, tag="rc", name="rc")
                    nc.vector.reciprocal(rc[:], po[64:65, :])
                    bc = small.tile([DKV, QC], F16, tag="bc", name="bc")
                    nc.gpsimd.partition_broadcast(bc[:], rc[:])
                    if h == 0:
                        dst = attnT01[0:DKV, ts(j, QC)]
                    elif h == 2:
                        dst = attnT2[:, ts(j, QC)]
                    else:
                        dst = None
                    if dst is not None:
                        nc.vector.tensor_mul(dst, po[0:64, :], bc[:])
                    else:
                        # head 1 lands at partitions 64:128 of the stacked
                        # pair tile; DVE can't shift partitions, so stage at
                        # base 0 and DMA-shift (SBUF->SBUF moves are
                        # partition-agnostic)
                        stg = small.tile([DKV, QC], F16, tag="stg", name="stg")
                        nc.vector.tensor_mul(stg[:], po[0:64, :], bc[:])
                        nc.sync.dma_start(attnT01[DKV:128, ts(j, QC)], stg[:])
                return norm

            def make_wo(i, tail=False):
                # Wo partial for q-tile i (128 q rows)
                def wo_i():
                    ot = outp.tile([128, D], F16, tag="ot", name="ot")
                    for half in range(2):
                        if tail:
                            pool, tag = (ps, "ps") if half else (projps, "pp")
                            pf = pool.tile([128, 384], F32, tag=tag, name="pf")
                        else:
                            pf = psF.tile([128, 384], F32, tag="pf", name="pf")
                        hs = slice(384 * half, 384 * half + 384)
                        nc.tensor.matmul(
                            pf[:], attnT01[:, ts(i, KT)], wo01_sb[:, hs],
                            start=True, stop=False,
                        )
                        nc.tensor.matmul(
                            pf[:], attnT2[:, ts(i, KT)], wo2_sb[:, hs],
                            start=False, stop=True,
                        )
                        if half == 0:
                            nc.vector.tensor_copy(ot[:, 0:384], pf[:])
                        else:
                            nc.scalar.copy(ot[:, 384:768], pf[:])
                    nc.sync.dma_start(out[i], ot[:])
                return wo_i

            # Software pipelining via two drip queues:
            #  - normq: deferred normalization + Wo closures (FIFO keeps
            #    norm(h,j) ahead of wo(j,*) which reads normalized attnT);
            #    popped at (h,j) / tt boundaries so po slots recycle.
            #  - projq: projection units for q-chunk j+1 (Q/K chunk matmuls,
            #    V k-tiles), dripped one per t-step of attention(j) so the
            #    serial projection phase disappears into attention's PE gaps.
            normq = []
            projq = []
            n_bt_dmas = [0]
            # chunk-0 projections must precede attention(0); attention is not
            # running yet, so spread them over the idle pair-pool psum slots
            # to pipeline instead of serializing on the single "pp" slot
            startup_slots = [(projps, "pp"), (ps, "ps"), (psO, "po")]
            for g in range(len(proj_groups)):
                make_qk_unit(0, g, startup_slots[g % 3])()
            for t in range(4):
                make_v_unit(t, startup_slots[t % 3])()

            for j in range(NQC):
                if j + 1 < NQC:
                    for g in range(len(proj_groups)):
                        projq.append(make_qk_unit(j + 1, g))
                    for t in range(4 * (j + 1), 4 * (j + 1) + 4):
                        projq.append(make_v_unit(t))
                # drip projections evenly: 8 units over this j's 12(j+1)
                # t-steps, front-loaded enough to finish before attn(j+1)
                stride = max(1, (6 * (j + 1)) // 9)
                slot = 0
                for h in range(HPC):
                    # free po slots before claiming one (po bufs=2)
                    for _ in range(min(2, len(normq))):
                        normq.pop(0)()
                    po = psO.tile([65, QC], F32, tag="po")
                    nkt = 4 * j + 4  # causal: k-tiles 0..4j+3
                    for tt in range(j + 1):  # batched expb DMA: 4 k-tiles
                        if normq:
                            normq.pop(0)()
                        bt = biasp.tile([128, 4, QC], F16, tag="bt")
                        bt_dma = nc.sync.dma_start(
                            bt[:],
                            expb[h, 4 * tt:4 * tt + 4, :, ts(j, QC)]
                            .rearrange("t p q -> p t q"),
                        )
                        if n_bt_dmas[0] < 4:
                            # don't let early expb prefetch steal HBM
                            # bandwidth from the critical-path xT load
                            add_dep_helper(
                                bt_dma.ins, xT_dmas[-1].ins,
                                reason="expb prefetch after xT",
                            )
                        n_bt_dmas[0] += 1
                        for pr in range(2):  # two k-tile pairs per tt-group
                            pss = ps.tile([128, 2, QC], F32, tag="ps")
                            for half in range(2):
                                t = 4 * tt + 2 * pr + half
                                k_sl, q_sl = qk_slices(h, t, j)
                                nc.tensor.matmul(
                                    pss[:, half, :], k_sl, q_sl,
                                    start=True, stop=True,
                                )
                            es = expsp.tile([128, 2, QC], F16, tag="es")
                            nc.scalar.activation(es[:], pss[:], EXP)
                            et = expp.tile([128, 2, QC], F16, tag="et")
                            nc.vector.tensor_mul(
                                et[:], es[:], bt[:, 2 * pr:2 * pr + 2, :]
                            )
                            for half in range(2):
                                t = 4 * tt + 2 * pr + half
                                nc.tensor.matmul(
                                    po[:], v1[:, t, h, :], et[:, half, :],
                                    start=(t == 0), stop=(t == nkt - 1),
                                )
                            if projq and slot % stride == 0:
                                projq.pop(0)()
                            slot += 1
                    normq.append(make_norm(po, h, j))
                for i in range(4 * j, 4 * j + 4):
                    normq.append(make_wo(i, tail=(j == NQC - 1)))
            for fn in normq + projq:
                fn()

    nc.compile()
    return nc


def get_program():
    if "nc" not in _prog_cache:
        _prog_cache["nc"] = _build_program()
    return _prog_cache["nc"]


def make_in_maps(hidden_states, attention_mask, position_bias, Wq, Wk, Wv, Wo):
    hs = np.ascontiguousarray(np.asarray(hidden_states, dtype=np.float32))
    am = np.asarray(attention_mask, dtype=np.float32)
    pb = np.asarray(position_bias, dtype=np.float32)
    wq = np.asarray(Wq, dtype=np.float32) * np.float32(1.0 / np.sqrt(DKV))
    wk = np.asarray(Wk, dtype=np.float32)
    wv_ = np.asarray(Wv, dtype=np.float32)
    wo_ = np.asarray(Wo, dtype=np.float32)

    # causal addend in [k, q] indexing: NEG where k > q
    kk = np.arange(Q, dtype=np.int64)
    causal_T = np.where(kk[:, None] > kk[None, :], np.float32(NEG), np.float32(0.0))
    causal_T = causal_T.astype(np.float32)

    in_maps = []
    for core in range(NCORES):
        b, g = divmod(core, NCORES // B)
        h0 = g * HPC
        # X^T chunked: [128, DC, Q], [p, c, q] = hs[b, q, 128c+p]
        xT = np.ascontiguousarray(
            hs[b].T.reshape(DC, 128, Q).transpose(1, 0, 2)
        ).astype(np.float16)
        # wqk: [128, DC, 384]: [Wq'01 | Wk01 | Wq'2 | Wk2]
        wq_sl = wq[:, h0 * DKV:(h0 + HPC) * DKV]
        wk_sl = wk[:, h0 * DKV:(h0 + HPC) * DKV]
        wqk = np.concatenate([
            wq_sl[:, 0:128], wk_sl[:, 0:128],
            wq_sl[:, 128:192], wk_sl[:, 128:192],
        ], axis=1)  # (D, 384)
        wqk = np.ascontiguousarray(
            wqk.reshape(DC, 128, 2 * HPC * DKV).transpose(1, 0, 2)
        ).astype(np.float16)
        # wv: [128, DC, HPC*DKV]
        wv_sl = wv_[:, (h0) * DKV:(h0 + HPC) * DKV].reshape(DC, 128, HPC * DKV)
        wv_sl = np.ascontiguousarray(wv_sl.transpose(1, 0, 2)).astype(np.float16)
        # wo: [DKV, HPC, D]: [p, h, n] = Wo[(h0+h)*DKV + p, n]
        wo_sl = np.ascontiguousarray(
            wo_[h0 * DKV:(h0 + HPC) * DKV, :]
        ).astype(np.float16)
        # expb: [HPC, NKT, 128, Q]: exp(biasT + causal + mask_k); masked -> 0
        # (attention_mask indexes k, which is the row dim of the transposed
        # bias, so it folds in as a per-row addend before the exp)
        bT = pb[0, h0:h0 + HPC].transpose(0, 2, 1) + causal_T[None]
        bT += am[b, 0, 0][None, :, None]
        bT = np.exp(bT, out=bT)
        bT = np.ascontiguousarray(bT.reshape(HPC, NKT, 128, Q)).astype(np.float16)
        in_maps.append({
            "xT": xT, "wqk": wqk, "wv": wv_sl, "wo": wo_sl,
            "expb": bT,
        })
    return in_maps


def kernel(hidden_states, attention_mask, position_bias, Wq, Wk, Wv, Wo):
    from concourse.bass_utils import run_bass_kernel_spmd

    nc = get_program()
    in_maps = make_in_maps(
        hidden_states, attention_mask, position_bias, Wq, Wk, Wv, Wo
    )
    res = run_bass_kernel_spmd(nc, in_maps, list(range(NCORES)))
    out = np.zeros((B, Q, D), dtype=np.float32)
    for core in range(NCORES):
        b = core // (NCORES // B)
        out[b] += res.results[core]["out"].reshape(Q, D).astype(np.float32)
    return out


# revision 58
# speedup vs baseline: 1.0804x; 1.0036x over previous
"""Bass/Tile TRN2 kernel for nn_MultiHeadAttention_4329327034628.

Multi-head self-attention with additive position bias + causal mask
(T5-style), B=2, Q=2048, D=768, H=12, DKV=64, fp32.

Sharding over 8 NeuronCores: core k -> (batch b = k//4, head-group
g = k%4 of 3 heads).  Each core computes its heads' attention and a
partial output projection (attn @ Wo_slice); the host sums the 4
partials per batch (the post-Wo all-reduce done at gather time).

Device-side layout strategy (no on-chip transposes needed):
  - host ships X^T (D on partitions) -> QKV projections contract D.
  - Q^T, K^T kept as [dkv, q]; scores computed transposed:
      scores^T [k, q] = lhsT(K^T slice).T @ rhs(Q^T)   (contract dkv)
  - position_bias is pre-transposed on host to [k, q] tiles, the causal
    NEG added, and *exponentiated* (expb = exp(biasT + causal), fp16):
    exp(s + b) = exp(s) * exp(b), so the device does ACT exp(s) followed
    by a cheap fp16 2x-mode DVE multiply -- no fp32 PSUM add needed.
    Masked entries have expb == 0 exactly -> probs match the reference.
  - attention_mask indexes k = partitions -> fused into the Exp
    activation as a per-partition bias.
  - softmax without max-subtraction (scores bounded by ~ +-10).
  - row-sum of exp fused into the AV matmul via a ones column:
      lhsT = [V_h | 1] [128k, 65] -> out rows 0..63 = out^T, row 64 = sum.
  - normalization: recip(sum) broadcast via ones-matmul, DVE multiply.
  - Wo: lhsT = attnT_h [64, 128q], rhs = Wo slice [64, 384] -> natural
    [q, D] partial output, DMA'd out.
"""

import numpy as np

B, Q, D, H, DKV = 2, 2048, 768, 12, 64
HPC = 3              # heads per core
NCORES = 8
NEG = -30000.0       # causal mask addend; exp(x + NEG) == 0.0 in fp32
QC = 512             # q chunk (moving dim)
KT = 128             # k tile (partition dim)
NQC = Q // QC        # 4
NKT = Q // KT        # 16
DC = D // 128        # 6 contraction chunks

_prog_cache = {}


def _build_program():
    import concourse.bass as bass
    import concourse.tile as tile
    from concourse import bacc, mybir
    from concourse.bass import ts

    F32 = mybir.dt.float32
    F16 = mybir.dt.float16
    EXP = mybir.ActivationFunctionType.Exp

    nc = bacc.Bacc("TRN2", target_bir_lowering=False, debug=False)

    xT = nc.dram_tensor("xT", [128, DC, Q], F16, kind="ExternalInput").ap()
    # cols 0:128 = Wq' heads {0,1}; 128:256 = Wk heads {0,1};
    # 256:320 = Wq' head 2; 320:384 = Wk head 2
    wqk = nc.dram_tensor("wqk", [128, DC, 2 * HPC * DKV], F16, kind="ExternalInput").ap()
    wv = nc.dram_tensor("wv", [128, DC, HPC * DKV], F16, kind="ExternalInput").ap()
    wo = nc.dram_tensor("wo", [HPC * DKV, D], F16, kind="ExternalInput").ap()
    expb = nc.dram_tensor("expb", [HPC, NKT, 128, Q], F16, kind="ExternalInput").ap()
    out = nc.dram_tensor("out", [NKT, 128, D], F16, kind="ExternalOutput").ap()

    with tile.TileContext(nc) as tc:
        with (
            nc.allow_low_precision(reason="fp16 matmul operands; fp32 psum accum"),
            tc.tile_pool(name="const", bufs=1) as const,
            tc.tile_pool(name="ps", bufs=2, space="PSUM") as ps,
            tc.tile_pool(name="projps", bufs=1, space="PSUM") as projps,
            tc.tile_pool(name="psO", bufs=2, space="PSUM") as psO,
            tc.tile_pool(name="psF", bufs=1, space="PSUM") as psF,
            tc.tile_pool(name="biasp", bufs=5) as biasp,
            tc.tile_pool(name="expsp", bufs=6) as expsp,
            tc.tile_pool(name="expp", bufs=6) as expp,
            tc.tile_pool(name="small", bufs=3) as small,
            tc.tile_pool(name="outp", bufs=3) as outp,
        ):
            # ---- stage A: load everything ----
            from concourse.tile import add_dep_helper
            wqk_sb = const.tile([128, DC, 2 * HPC * DKV], F16, tag="wqk")
            for c in range(DC):
                nc.sync.dma_start(wqk_sb[:, c, :], wqk[:, c, :])
            wv_sb = const.tile([128, DC, HPC * DKV], F16, tag="wv")
            nc.sync.dma_start(wv_sb[:], wv[:])
            # Wo stacked: [0:128] = heads {0,1} vertically, wo2 = head 2
            wo01_sb = const.tile([2 * DKV, D], F16, tag="wo01")
            nc.sync.dma_start(wo01_sb[:], wo[0:2 * DKV, :])
            wo2_sb = const.tile([DKV, D], F16, tag="wo2")
            nc.sync.dma_start(wo2_sb[:], wo[2 * DKV:, :])
            xT_sb = const.tile([128, DC, Q], F16, tag="xT")
            xT_dmas = [
                nc.sync.dma_start(
                    xT_sb[:, c, ts(hf, Q // 2)], xT[:, c, ts(hf, Q // 2)]
                )
                for c in range(DC)
                for hf in range(2)
            ]
            ones1 = const.tile([1, DKV], F16, tag="ones1")
            nc.gpsimd.memset(ones1[:], 1.0)

            # ---- stage B: projections ----
            # Q^T / K^T storage. Heads 0,1 paired in [128, Q] tiles (head 0 =
            # rows 0:64, head 1 = rows 64:128, so scores-matmul operands share
            # a base partition); head 2 in separate [64, Q] tiles (base 0).
            qT01 = const.tile([128, Q], F16, tag="qT01")
            kT01 = const.tile([128, Q], F16, tag="kT01")
            # head 2: one M=128 group -> qkT2 rows 0:64 = Q^T, 64:128 = K^T;
            # K^T is then DMA-shifted down to kT2b rows 0:64 so the scores
            # matmul operands share base partition 0
            qkT2 = const.tile([128, Q], F16, tag="qkT2")
            kT2b = const.tile([DKV, Q], F16, tag="kT2b")
            # (lhsT weight slice, dest tile) per projection matmul group
            proj_groups = [
                ((0, 128), qT01), ((128, 256), kT01), ((256, 384), qkT2),
            ]
            def make_qk_unit(j, g, pool_tag=None):
                (w0, w1), dst = proj_groups[g]

                def qk_unit():
                    pool, tag = pool_tag or (projps, "pp")
                    p = pool.tile([w1 - w0, QC], F32, tag=tag, name="p")
                    for c in range(DC):
                        nc.tensor.matmul(
                            p[:], wqk_sb[:, c, w0:w1], xT_sb[:, c, ts(j, QC)],
                            start=(c == 0), stop=(c == DC - 1),
                        )
                    nc.scalar.copy(dst[:, ts(j, QC)], p[:])
                    if dst is qkT2:
                        nc.sync.dma_start(
                            kT2b[:, ts(j, QC)], qkT2[DKV:128, ts(j, QC)]
                        )
                return qk_unit

            def qk_slices(h, t, j):
                """(lhsT k-slice, rhs q-slice) for head h, k-tile t, q-chunk j."""
                if h == 0:
                    return kT01[0:DKV, ts(t, KT)], qT01[0:DKV, ts(j, QC)]
                if h == 1:
                    return kT01[DKV:128, ts(t, KT)], qT01[DKV:128, ts(j, QC)]
                return kT2b[:, ts(t, KT)], qkT2[0:DKV, ts(j, QC)]

            # V (natural [k, d]) with a ones column per head: [128, NKT, 3, 65]
            # per (tile t, head h): v1[:, t, h, 0:64] = V_h, v1[:, t, h, 64] = 1
            v1 = const.tile([128, NKT, HPC, DKV + 1], F16, tag="v1")
            nc.gpsimd.memset(v1[:], 1.0)

            def make_v_unit(t, pool_tag=None):
                def v_unit():
                    pool, tag = pool_tag or (projps, "pp")
                    pv = pool.tile([128, HPC * DKV], F32, tag=tag, name="pv")
                    for c in range(DC):
                        nc.tensor.matmul(
                            pv[:], xT_sb[:, c, ts(t, KT)], wv_sb[:, c, :],
                            start=(c == 0), stop=(c == DC - 1),
                        )
                    # single strided copy: [128, 3, 64] <- [128, (3 64)]
                    nc.vector.tensor_copy(
                        v1[:, t, :, 0:DKV],
                        pv[:].rearrange("p (h d) -> p h d", h=HPC),
                    )
                return v_unit

            # ---- stage C: attention (scores^T layout), stage D: Wo ----
            attnT01 = const.tile([2 * DKV, Q], F16, tag="attnT01")
            attnT2 = const.tile([DKV, Q], F16, tag="attnT2")
            def make_norm(po, h, j):
                # normalize: attnT_h[:, jq] = po[0:64] * (1/po[64]) bcast
                def norm():
                    rc = small.tile([1, QC], F16, tag="rc", name="rc")
                    nc.vector.reciprocal(rc[:], po[64:65, :])
                    bc = small.tile([DKV, QC], F16, tag="bc", name="bc")
                    nc.gpsimd.partition_broadcast(bc[:], rc[:])
                    if h == 0:
                        dst = attnT01[0:DKV, ts(j, QC)]
                    elif h == 2:
                        dst = attnT2[:, ts(j, QC)]
                    else:
                        dst = None
                    if dst is not None:
                        nc.vector.tensor_mul(dst, po[0:64, :], bc[:])
                    else:
                        # head 1 lands at partitions 64:128 of the stacked
                        # pair tile; DVE can't shift partitions, so stage at
                        # base 0 and DMA-shift (SBUF->SBUF moves are
                        # partition-agnostic)
                        stg = small.tile([DKV, QC], F16, tag="stg", name="stg")
                        nc.vector.tensor_mul(stg[:], po[0:64, :], bc[:])
                        nc.sync.dma_start(attnT01[DKV:128, ts(j, QC)], stg[:])
                return norm

            def make_wo(i, tail=False):
                # Wo partial for q-tile i (128 q rows)
                def wo_i():
                    ot = outp.tile([128, D], F16, tag="ot", name="ot")
                    for half in range(2):
                        if tail:
                            pool, tag = (ps, "ps") if half else (projps, "pp")
                            pf = pool.tile([128, 384], F32, tag=tag, name="pf")
                        else:
                            pf = psF.tile([128, 384], F32, tag="pf", name="pf")
                        hs = slice(384 * half, 384 * half + 384)
                        nc.tensor.matmul(
                            pf[:], attnT01[:, ts(i, KT)], wo01_sb[:, hs],
                            start=True, stop=False,
                        )
                        nc.tensor.matmul(
                            pf[:], attnT2[:, ts(i, KT)], wo2_sb[:, hs],
                            start=False, stop=True,
                        )
                        if half == 0:
                            nc.vector.tensor_copy(ot[:, 0:384], pf[:])
                        elif i < 8:
                            # early chunks: ACT has slack; late chunks are
                            # ACT-bound (exp), keep copies off its queue
                            nc.scalar.copy(ot[:, 384:768], pf[:])
                        else:
                            nc.vector.tensor_copy(ot[:, 384:768], pf[:])
                    nc.sync.dma_start(out[i], ot[:])
                return wo_i

            # Software pipelining via two drip queues:
            #  - normq: deferred normalization + Wo closures (FIFO keeps
            #    norm(h,j) ahead of wo(j,*) which reads normalized attnT);
            #    popped at (h,j) / tt boundaries so po slots recycle.
            #  - projq: projection units for q-chunk j+1 (Q/K chunk matmuls,
            #    V k-tiles), dripped one per t-step of attention(j) so the
            #    serial projection phase disappears into attention's PE gaps.
            normq = []
            projq = []
            n_bt_dmas = [0]
            # chunk-0 projections must precede attention(0); attention is not
            # running yet, so spread them over the idle pair-pool psum slots
            # to pipeline instead of serializing on the single "pp" slot
            startup_slots = [(projps, "pp"), (ps, "ps"), (psO, "po")]
            for g in range(len(proj_groups)):
                make_qk_unit(0, g, startup_slots[g % 3])()
            for t in range(4):
                make_v_unit(t, startup_slots[t % 3])()

            for j in range(NQC):
                # any leftover proj(j) units must be emitted before
                # attention(j) reads their outputs
                while projq:
                    projq.pop(0)()
                if j + 1 < NQC:
                    for g in range(len(proj_groups)):
                        projq.append(make_qk_unit(j + 1, g))
                    for t in range(4 * (j + 1), 4 * (j + 1) + 4):
                        projq.append(make_v_unit(t))
                # drip projections evenly: 8 units over this j's 12(j+1)
                # t-steps, front-loaded enough to finish before attn(j+1)
                stride = max(1, (6 * (j + 1)) // 9)
                slot = 0
                for h in range(HPC):
                    # free po slots before claiming one (po bufs=2)
                    for _ in range(min(2, len(normq))):
                        normq.pop(0)()
                    po = psO.tile([65, QC], F32, tag="po")
                    nkt = 4 * j + 4  # causal: k-tiles 0..4j+3
                    for tt in range(j + 1):  # batched expb DMA: 4 k-tiles
                        if normq:
                            normq.pop(0)()
                        bt = biasp.tile([128, 4, QC], F16, tag="bt")
                        bt_dma = nc.sync.dma_start(
                            bt[:],
                            expb[h, 4 * tt:4 * tt + 4, :, ts(j, QC)]
                            .rearrange("t p q -> p t q"),
                        )
                        if n_bt_dmas[0] < 4:
                            # don't let early expb prefetch steal HBM
                            # bandwidth from the critical-path xT load
                            add_dep_helper(
                                bt_dma.ins, xT_dmas[-1].ins,
                                reason="expb prefetch after xT",
                            )
                        n_bt_dmas[0] += 1
                        for pr in range(2):  # two k-tile pairs per tt-group
                            pss = ps.tile([128, 2, QC], F32, tag="ps")
                            for half in range(2):
                                t = 4 * tt + 2 * pr + half
                                k_sl, q_sl = qk_slices(h, t, j)
                                nc.tensor.matmul(
                                    pss[:, half, :], k_sl, q_sl,
                                    start=True, stop=True,
                                )
                            es = expsp.tile([128, 2, QC], F16, tag="es")
                            nc.scalar.activation(es[:], pss[:], EXP)
                            et = expp.tile([128, 2, QC], F16, tag="et")
                            nc.vector.tensor_mul(
                                et[:], es[:], bt[:, 2 * pr:2 * pr + 2, :]
                            )
                            for half in range(2):
                                t = 4 * tt + 2 * pr + half
                                nc.tensor.matmul(
                                    po[:], v1[:, t, h, :], et[:, half, :],
                                    start=(t == 0), stop=(t == nkt - 1),
                                )
                            if projq and slot % stride == 0:
                                projq.pop(0)()
                            slot += 1
                    normq.append(make_norm(po, h, j))
                for i in range(4 * j, 4 * j + 4):
                    normq.append(make_wo(i, tail=(j == NQC - 1)))
            for fn in normq + projq:
                fn()

    nc.compile()
    return nc


def get_program():
    if "nc" not in _prog_cache:
        _prog_cache["nc"] = _build_program()
    return _prog_cache["nc"]


def make_in_maps(hidden_states, attention_mask, position_bias, Wq, Wk, Wv, Wo):
    hs = np.ascontiguousarray(np.asarray(hidden_states, dtype=np.float32))
    am = np.asarray(attention_mask, dtype=np.float32)
    pb = np.asarray(position_bias, dtype=np.float32)
    wq = np.asarray(Wq, dtype=np.float32) * np.float32(1.0 / np.sqrt(DKV))
    wk = np.asarray(Wk, dtype=np.float32)
    wv_ = np.asarray(Wv, dtype=np.float32)
    wo_ = np.asarray(Wo, dtype=np.float32)

    # causal addend in [k, q] indexing: NEG where k > q
    kk = np.arange(Q, dtype=np.int64)
    causal_T = np.where(kk[:, None] > kk[None, :], np.float32(NEG), np.float32(0.0))
    causal_T = causal_T.astype(np.float32)

    in_maps = []
    for core in range(NCORES):
        b, g = divmod(core, NCORES // B)
        h0 = g * HPC
        # X^T chunked: [128, DC, Q], [p, c, q] = hs[b, q, 128c+p]
        xT = np.ascontiguousarray(
            hs[b].T.reshape(DC, 128, Q).transpose(1, 0, 2)
        ).astype(np.float16)
        # wqk: [128, DC, 384]: [Wq'01 | Wk01 | Wq'2 | Wk2]
        wq_sl = wq[:, h0 * DKV:(h0 + HPC) * DKV]
        wk_sl = wk[:, h0 * DKV:(h0 + HPC) * DKV]
        wqk = np.concatenate([
            wq_sl[:, 0:128], wk_sl[:, 0:128],
            wq_sl[:, 128:192], wk_sl[:, 128:192],
        ], axis=1)  # (D, 384)
        wqk = np.ascontiguousarray(
            wqk.reshape(DC, 128, 2 * HPC * DKV).transpose(1, 0, 2)
        ).astype(np.float16)
        # wv: [128, DC, HPC*DKV]
        wv_sl = wv_[:, (h0) * DKV:(h0 + HPC) * DKV].reshape(DC, 128, HPC * DKV)
        wv_sl = np.ascontiguousarray(wv_sl.transpose(1, 0, 2)).astype(np.float16)
        # wo: [DKV, HPC, D]: [p, h, n] = Wo[(h0+h)*DKV + p, n]
        wo_sl = np.ascontiguousarray(
            wo_[h0 * DKV:(h0 + HPC) * DKV, :]
        ).astype(np.float16)
        # expb: [HPC, NKT, 128, Q]: exp(biasT + causal + mask_k); masked -> 0
        # (attention_mask indexes k, which is the row dim of the transposed
        # bias, so it folds in as a per-row addend before the exp)
        bT = pb[0, h0:h0 + HPC].transpose(0, 2, 1) + causal_T[None]
        bT += am[b, 0, 0][None, :, None]
        bT = np.exp(bT, out=bT)
        bT = np.ascontiguousarray(bT.reshape(HPC, NKT, 128, Q)).astype(np.float16)
        in_maps.append({
            "xT": xT, "wqk": wqk, "wv": wv_sl, "wo": wo_sl,
            "expb": bT,
        })
    return in_maps


def kernel(hidden_states, attention_mask, position_bias, Wq, Wk, Wv, Wo):
    from concourse.bass_utils import run_bass_kernel_spmd

    nc = get_program()
    in_maps = make_in_maps(
        hidden_states, attention_mask, position_bias, Wq, Wk, Wv, Wo
    )
    res = run_bass_kernel_spmd(nc, in_maps, list(range(NCORES)))
    out = np.zeros((B, Q, D), dtype=np.float32)
    for core in range(NCORES):
        b = core // (NCORES // B)
        out[b] += res.results[core]["out"].reshape(Q, D).astype(np.float32)
    return out


# revision 65
# speedup vs baseline: 1.1004x; 1.0185x over previous
"""Bass/Tile TRN2 kernel for nn_MultiHeadAttention_4329327034628.

Multi-head self-attention with additive position bias + causal mask
(T5-style), B=2, Q=2048, D=768, H=12, DKV=64, fp32.

Sharding over 8 NeuronCores: core k -> (batch b = k//4, head-group
g = k%4 of 3 heads).  Each core computes its heads' attention and a
partial output projection (attn @ Wo_slice); the host sums the 4
partials per batch (the post-Wo all-reduce done at gather time).

Device-side layout strategy (no on-chip transposes needed):
  - host ships X^T (D on partitions) -> QKV projections contract D.
  - Q^T, K^T kept as [dkv, q]; scores computed transposed:
      scores^T [k, q] = lhsT(K^T slice).T @ rhs(Q^T)   (contract dkv)
  - position_bias is pre-transposed on host to [k, q] tiles, the causal
    NEG added, and *exponentiated* (expb = exp(biasT + causal), fp16):
    exp(s + b) = exp(s) * exp(b), so the device does ACT exp(s) followed
    by a cheap fp16 2x-mode DVE multiply -- no fp32 PSUM add needed.
    Masked entries have expb == 0 exactly -> probs match the reference.
  - attention_mask indexes k = partitions -> fused into the Exp
    activation as a per-partition bias.
  - softmax without max-subtraction (scores bounded by ~ +-10).
  - row-sum of exp fused into the AV matmul via a ones column:
      lhsT = [V_h | 1] [128k, 65] -> out rows 0..63 = out^T, row 64 = sum.
  - normalization: recip(sum) broadcast via ones-matmul, DVE multiply.
  - Wo: lhsT = attnT_h [64, 128q], rhs = Wo slice [64, 384] -> natural
    [q, D] partial output, DMA'd out.
"""

import numpy as np

B, Q, D, H, DKV = 2, 2048, 768, 12, 64
HPC = 3              # heads per core
NCORES = 8
NEG = -30000.0       # causal mask addend; exp(x + NEG) == 0.0 in fp32
QC = 512             # q chunk (moving dim)
KT = 128             # k tile (partition dim)
NQC = Q // QC        # 4
NKT = Q // KT        # 16
DC = D // 128        # 6 contraction chunks

_prog_cache = {}


def _build_program():
    import concourse.bass as bass
    import concourse.tile as tile
    from concourse import bacc, mybir
    from concourse.bass import ts

    F32 = mybir.dt.float32
    F16 = mybir.dt.float16
    EXP = mybir.ActivationFunctionType.Exp

    nc = bacc.Bacc("TRN2", target_bir_lowering=False, debug=False)

    xT = nc.dram_tensor("xT", [128, DC, Q], F16, kind="ExternalInput").ap()
    # cols 0:128 = Wq' heads {0,1}; 128:256 = Wk heads {0,1};
    # 256:320 = Wq' head 2; 320:384 = Wk head 2
    wqk = nc.dram_tensor("wqk", [128, DC, 2 * HPC * DKV], F16, kind="ExternalInput").ap()
    wv = nc.dram_tensor("wv", [128, DC, HPC * DKV], F16, kind="ExternalInput").ap()
    wo = nc.dram_tensor("wo", [HPC * DKV, D], F16, kind="ExternalInput").ap()
    expb = nc.dram_tensor("expb", [HPC, NKT, 128, Q], F16, kind="ExternalInput").ap()
    out = nc.dram_tensor("out", [NKT, 128, D], F16, kind="ExternalOutput").ap()

    with tile.TileContext(nc) as tc:
        with (
            nc.allow_low_precision(reason="fp16 matmul operands; fp32 psum accum"),
            tc.tile_pool(name="const", bufs=1) as const,
            tc.tile_pool(name="ps", bufs=2, space="PSUM") as ps,
            tc.tile_pool(name="projps", bufs=1, space="PSUM") as projps,
            tc.tile_pool(name="psO", bufs=2, space="PSUM") as psO,
            tc.tile_pool(name="psF", bufs=1, space="PSUM") as psF,
            tc.tile_pool(name="biasp", bufs=5) as biasp,
            tc.tile_pool(name="expsp", bufs=6) as expsp,
            tc.tile_pool(name="expp", bufs=6) as expp,
            tc.tile_pool(name="small", bufs=3) as small,
            tc.tile_pool(name="outp", bufs=3) as outp,
        ):
            # ---- stage A: load everything ----
            from concourse.tile import add_dep_helper
            wqk_sb = const.tile([128, DC, 2 * HPC * DKV], F16, tag="wqk")
            for c in range(DC):
                nc.sync.dma_start(wqk_sb[:, c, :], wqk[:, c, :])
            wv_sb = const.tile([128, DC, HPC * DKV], F16, tag="wv")
            nc.sync.dma_start(wv_sb[:], wv[:])
            # Wo stacked: [0:128] = heads {0,1} vertically, wo2 = head 2
            wo01_sb = const.tile([2 * DKV, D], F16, tag="wo01")
            nc.sync.dma_start(wo01_sb[:], wo[0:2 * DKV, :])
            wo2_sb = const.tile([DKV, D], F16, tag="wo2")
            nc.sync.dma_start(wo2_sb[:], wo[2 * DKV:, :])
            xT_sb = const.tile([128, DC, Q], F16, tag="xT")
            xT_dmas = [
                nc.sync.dma_start(
                    xT_sb[:, c, ts(hf, Q // 2)], xT[:, c, ts(hf, Q // 2)]
                )
                for c in range(DC)
                for hf in range(2)
            ]
            ones1 = const.tile([1, DKV], F16, tag="ones1")
            nc.gpsimd.memset(ones1[:], 1.0)

            # ---- stage B: projections ----
            # Q^T / K^T storage. Heads 0,1 paired in [128, Q] tiles (head 0 =
            # rows 0:64, head 1 = rows 64:128, so scores-matmul operands share
            # a base partition); head 2 in separate [64, Q] tiles (base 0).
            qT01 = const.tile([128, Q], F16, tag="qT01")
            kT01 = const.tile([128, Q], F16, tag="kT01")
            # head 2: one M=128 group -> qkT2 rows 0:64 = Q^T, 64:128 = K^T;
            # K^T is then DMA-shifted down to kT2b rows 0:64 so the scores
            # matmul operands share base partition 0
            qkT2 = const.tile([128, Q], F16, tag="qkT2")
            kT2b = const.tile([DKV, Q], F16, tag="kT2b")
            # (lhsT weight slice, dest tile) per projection matmul group
            proj_groups = [
                ((0, 128), qT01), ((128, 256), kT01), ((256, 384), qkT2),
            ]
            def make_qk_unit(j, g, pool_tag=None):
                (w0, w1), dst = proj_groups[g]

                def qk_unit():
                    pool, tag = pool_tag or (projps, "pp")
                    p = pool.tile([w1 - w0, QC], F32, tag=tag, name="p")
                    for c in range(DC):
                        nc.tensor.matmul(
                            p[:], wqk_sb[:, c, w0:w1], xT_sb[:, c, ts(j, QC)],
                            start=(c == 0), stop=(c == DC - 1),
                        )
                    nc.scalar.copy(dst[:, ts(j, QC)], p[:])
                    if dst is qkT2:
                        nc.sync.dma_start(
                            kT2b[:, ts(j, QC)], qkT2[DKV:128, ts(j, QC)]
                        )
                return qk_unit

            def qk_slices(h, t, j):
                """(lhsT k-slice, rhs q-slice) for head h, k-tile t, q-chunk j."""
                if h == 0:
                    return kT01[0:DKV, ts(t, KT)], qT01[0:DKV, ts(j, QC)]
                if h == 1:
                    return kT01[DKV:128, ts(t, KT)], qT01[DKV:128, ts(j, QC)]
                return kT2b[:, ts(t, KT)], qkT2[0:DKV, ts(j, QC)]

            # V (natural [k, d]) with a ones column per head: [128, NKT, 3, 65]
            # per (tile t, head h): v1[:, t, h, 0:64] = V_h, v1[:, t, h, 64] = 1
            v1 = const.tile([128, NKT, HPC, DKV + 1], F16, tag="v1")
            nc.gpsimd.memset(v1[:], 1.0)

            def make_v_unit(t, pool_tag=None):
                def v_unit():
                    pool, tag = pool_tag or (projps, "pp")
                    pv = pool.tile([128, HPC * DKV], F32, tag=tag, name="pv")
                    for c in range(DC):
                        nc.tensor.matmul(
                            pv[:], xT_sb[:, c, ts(t, KT)], wv_sb[:, c, :],
                            start=(c == 0), stop=(c == DC - 1),
                        )
                    # single strided copy: [128, 3, 64] <- [128, (3 64)]
                    nc.vector.tensor_copy(
                        v1[:, t, :, 0:DKV],
                        pv[:].rearrange("p (h d) -> p h d", h=HPC),
                    )
                return v_unit

            # ---- stage C: attention (scores^T layout), stage D: Wo ----
            attnT01 = const.tile([2 * DKV, Q], F16, tag="attnT01")
            attnT2 = const.tile([DKV, Q], F16, tag="attnT2")
            def make_norm(po, h, j):
                # normalize: attnT_h[:, jq] = po[0:64] * (1/po[64]) bcast
                def norm():
                    rc = small.tile([1, QC], F16, tag="rc", name="rc")
                    nc.vector.reciprocal(rc[:], po[64:65, :])
                    bc = small.tile([DKV, QC], F16, tag="bc", name="bc")
                    nc.gpsimd.partition_broadcast(bc[:], rc[:])
                    if h == 0:
                        dst = attnT01[0:DKV, ts(j, QC)]
                    elif h == 2:
                        dst = attnT2[:, ts(j, QC)]
                    else:
                        dst = None
                    if dst is not None:
                        nc.vector.tensor_mul(dst, po[0:64, :], bc[:])
                    else:
                        # head 1 lands at partitions 64:128 of the stacked
                        # pair tile; DVE can't shift partitions, so stage at
                        # base 0 and DMA-shift (SBUF->SBUF moves are
                        # partition-agnostic)
                        stg = small.tile([DKV, QC], F16, tag="stg", name="stg")
                        nc.vector.tensor_mul(stg[:], po[0:64, :], bc[:])
                        nc.sync.dma_start(attnT01[DKV:128, ts(j, QC)], stg[:])
                return norm

            def make_wo(i, tail=False):
                # Wo partial for q-tile i (128 q rows)
                def wo_i():
                    ot = outp.tile([128, D], F16, tag="ot", name="ot")
                    for half in range(2):
                        if tail:
                            pool, tag = (ps, "ps") if half else (projps, "pp")
                            pf = pool.tile([128, 384], F32, tag=tag, name="pf")
                        else:
                            pf = psF.tile([128, 384], F32, tag="pf", name="pf")
                        hs = slice(384 * half, 384 * half + 384)
                        nc.tensor.matmul(
                            pf[:], attnT01[:, ts(i, KT)], wo01_sb[:, hs],
                            start=True, stop=False,
                        )
                        nc.tensor.matmul(
                            pf[:], attnT2[:, ts(i, KT)], wo2_sb[:, hs],
                            start=False, stop=True,
                        )
                        if half == 0:
                            nc.vector.tensor_copy(ot[:, 0:384], pf[:])
                        elif i < 8:
                            # early chunks: ACT has slack; late chunks are
                            # ACT-bound (exp), keep copies off its queue
                            nc.scalar.copy(ot[:, 384:768], pf[:])
                        else:
                            nc.vector.tensor_copy(ot[:, 384:768], pf[:])
                    nc.sync.dma_start(out[i], ot[:])
                return wo_i

            # Software pipelining via two drip queues:
            #  - normq: deferred normalization + Wo closures (FIFO keeps
            #    norm(h,j) ahead of wo(j,*) which reads normalized attnT);
            #    popped at (h,j) / tt boundaries so po slots recycle.
            #  - projq: projection units for q-chunk j+1 (Q/K chunk matmuls,
            #    V k-tiles), dripped one per t-step of attention(j) so the
            #    serial projection phase disappears into attention's PE gaps.
            normq = []
            projq = []
            n_bt_dmas = [0]
            # chunk-0 projections must precede attention(0); attention is not
            # running yet, so spread them over the idle pair-pool psum slots
            # to pipeline instead of serializing on the single "pp" slot
            startup_slots = [(projps, "pp"), (ps, "ps"), (psO, "po")]
            for g in range(len(proj_groups)):
                make_qk_unit(0, g, startup_slots[g % 3])()
            for t in range(4):
                make_v_unit(t, startup_slots[t % 3])()

            for j in range(NQC):
                # any leftover proj(j) units must be emitted before
                # attention(j) reads their outputs
                while projq:
                    projq.pop(0)()
                if j + 1 < NQC:
                    for g in range(len(proj_groups)):
                        projq.append(make_qk_unit(j + 1, g))
                    for t in range(4 * (j + 1), 4 * (j + 1) + 4):
                        projq.append(make_v_unit(t))
                # drip projections evenly: 8 units over this j's 12(j+1)
                # t-steps, front-loaded enough to finish before attn(j+1)
                stride = max(1, (6 * (j + 1)) // 9)
                slot = 0
                for h in range(HPC):
                    # free po slots before claiming one (po bufs=2)
                    for _ in range(min(2, len(normq))):
                        normq.pop(0)()
                    po = psO.tile([65, QC], F32, tag="po")
                    nkt = 4 * j + 4  # causal: k-tiles 0..4j+3
                    for tt in range(j + 1):  # batched expb DMA: 4 k-tiles
                        if normq:
                            normq.pop(0)()
                        bt = biasp.tile([128, 4, QC], F16, tag="bt")
                        if tt == j:
                            # diagonal group: second pair only needs the
                            # upper half of the q-chunk
                            bt_dma = nc.sync.dma_start(
                                bt[:, 0:2, :],
                                expb[h, 4 * tt:4 * tt + 2, :, ts(j, QC)]
                                .rearrange("t p q -> p t q"),
                            )
                            nc.sync.dma_start(
                                bt[:, 2:4, 256:],
                                expb[h, 4 * tt + 2:4 * tt + 4, :,
                                     512 * j + 256:512 * j + 512]
                                .rearrange("t p q -> p t q"),
                            )
                        else:
                            bt_dma = nc.sync.dma_start(
                                bt[:],
                                expb[h, 4 * tt:4 * tt + 4, :, ts(j, QC)]
                                .rearrange("t p q -> p t q"),
                            )
                        if n_bt_dmas[0] < 4:
                            # don't let early expb prefetch steal HBM
                            # bandwidth from the critical-path xT load
                            add_dep_helper(
                                bt_dma.ins, xT_dmas[-1].ins,
                                reason="expb prefetch after xT",
                            )
                        n_bt_dmas[0] += 1
                        for pr in range(2):  # two k-tile pairs per tt-group
                            # diagonal narrowing: in the last tt-group the
                            # second pair's tiles (k >= 512j+256) only see
                            # q >= 512j+256, so compute the upper half-chunk
                            # only (the skipped region is causally masked ->
                            # contributes exactly 0)
                            q0 = 256 if (tt == j and pr == 1) else 0
                            pss = ps.tile([128, 2, QC], F32, tag="ps")
                            for half in range(2):
                                t = 4 * tt + 2 * pr + half
                                k_sl, q_sl = qk_slices(h, t, j)
                                nc.tensor.matmul(
                                    pss[:, half, q0:], k_sl, q_sl[:, q0:],
                                    start=True, stop=True,
                                )
                            es = expsp.tile([128, 2, QC], F16, tag="es")
                            nc.scalar.activation(
                                es[:, :, q0:], pss[:, :, q0:], EXP
                            )
                            et = expp.tile([128, 2, QC], F16, tag="et")
                            nc.vector.tensor_mul(
                                et[:, :, q0:], es[:, :, q0:],
                                bt[:, 2 * pr:2 * pr + 2, q0:],
                            )
                            for half in range(2):
                                t = 4 * tt + 2 * pr + half
                                nc.tensor.matmul(
                                    po[:, q0:], v1[:, t, h, :],
                                    et[:, half, q0:],
                                    start=(t == 0), stop=(t == nkt - 1),
                                )
                            if projq and slot % stride == 0:
                                projq.pop(0)()
                            slot += 1
                    normq.append(make_norm(po, h, j))
                for i in range(4 * j, 4 * j + 4):
                    normq.append(make_wo(i, tail=(j == NQC - 1)))
            for fn in normq + projq:
                fn()

    nc.compile()
    return nc


def get_program():
    if "nc" not in _prog_cache:
        _prog_cache["nc"] = _build_program()
    return _prog_cache["nc"]


def make_in_maps(hidden_states, attention_mask, position_bias, Wq, Wk, Wv, Wo):
    hs = np.ascontiguousarray(np.asarray(hidden_states, dtype=np.float32))
    am = np.asarray(attention_mask, dtype=np.float32)
    pb = np.asarray(position_bias, dtype=np.float32)
    wq = np.asarray(Wq, dtype=np.float32) * np.float32(1.0 / np.sqrt(DKV))
    wk = np.asarray(Wk, dtype=np.float32)
    wv_ = np.asarray(Wv, dtype=np.float32)
    wo_ = np.asarray(Wo, dtype=np.float32)

    # causal addend in [k, q] indexing: NEG where k > q
    kk = np.arange(Q, dtype=np.int64)
    causal_T = np.where(kk[:, None] > kk[None, :], np.float32(NEG), np.float32(0.0))
    causal_T = causal_T.astype(np.float32)

    in_maps = []
    for core in range(NCORES):
        b, g = divmod(core, NCORES // B)
        h0 = g * HPC
        # X^T chunked: [128, DC, Q], [p, c, q] = hs[b, q, 128c+p]
        xT = np.ascontiguousarray(
            hs[b].T.reshape(DC, 128, Q).transpose(1, 0, 2)
        ).astype(np.float16)
        # wqk: [128, DC, 384]: [Wq'01 | Wk01 | Wq'2 | Wk2]
        wq_sl = wq[:, h0 * DKV:(h0 + HPC) * DKV]
        wk_sl = wk[:, h0 * DKV:(h0 + HPC) * DKV]
        wqk = np.concatenate([
            wq_sl[:, 0:128], wk_sl[:, 0:128],
            wq_sl[:, 128:192], wk_sl[:, 128:192],
        ], axis=1)  # (D, 384)
        wqk = np.ascontiguousarray(
            wqk.reshape(DC, 128, 2 * HPC * DKV).transpose(1, 0, 2)
        ).astype(np.float16)
        # wv: [128, DC, HPC*DKV]
        wv_sl = wv_[:, (h0) * DKV:(h0 + HPC) * DKV].reshape(DC, 128, HPC * DKV)
        wv_sl = np.ascontiguousarray(wv_sl.transpose(1, 0, 2)).astype(np.float16)
        # wo: [DKV, HPC, D]: [p, h, n] = Wo[(h0+h)*DKV + p, n]
        wo_sl = np.ascontiguousarray(
            wo_[h0 * DKV:(h0 + HPC) * DKV, :]
        ).astype(np.float16)
        # expb: [HPC, NKT, 128, Q]: exp(biasT + causal + mask_k); masked -> 0
        # (attention_mask indexes k, which is the row dim of the transposed
        # bias, so it folds in as a per-row addend before the exp)
        bT = pb[0, h0:h0 + HPC].transpose(0, 2, 1) + causal_T[None]
        bT += am[b, 0, 0][None, :, None]
        bT = np.exp(bT, out=bT)
        bT = np.ascontiguousarray(bT.reshape(HPC, NKT, 128, Q)).astype(np.float16)
        in_maps.append({
            "xT": xT, "wqk": wqk, "wv": wv_sl, "wo": wo_sl,
            "expb": bT,
        })
    return in_maps


def kernel(hidden_states, attention_mask, position_bias, Wq, Wk, Wv, Wo):
    from concourse.bass_utils import run_bass_kernel_spmd

    nc = get_program()
    in_maps = make_in_maps(
        hidden_states, attention_mask, position_bias, Wq, Wk, Wv, Wo
    )
    res = run_bass_kernel_spmd(nc, in_maps, list(range(NCORES)))
    out = np.zeros((B, Q, D), dtype=np.float32)
    for core in range(NCORES):
        b = core // (NCORES // B)
        out[b] += res.results[core]["out"].reshape(Q, D).astype(np.float32)
    return out


# revision 75
# speedup vs baseline: 1.1373x; 1.0335x over previous
"""Bass/Tile TRN2 kernel for nn_MultiHeadAttention_4329327034628.

Multi-head self-attention with additive position bias + causal mask
(T5-style), B=2, Q=2048, D=768, H=12, DKV=64, fp32.

Sharding over 8 NeuronCores: core k -> (batch b = k//4, head-group
g = k%4 of 3 heads).  Each core computes its heads' attention and a
partial output projection (attn @ Wo_slice); the host sums the 4
partials per batch (the post-Wo all-reduce done at gather time).

Device-side layout strategy (no on-chip transposes needed):
  - host ships X^T (D on partitions) -> QKV projections contract D.
  - Q^T, K^T kept as [dkv, q]; scores computed transposed:
      scores^T [k, q] = lhsT(K^T slice).T @ rhs(Q^T)   (contract dkv)
  - position_bias is pre-transposed on host to [k, q] tiles, the causal
    NEG added, and *exponentiated* (expb = exp(biasT + causal), fp16):
    exp(s + b) = exp(s) * exp(b), so the device does ACT exp(s) followed
    by a cheap fp16 2x-mode DVE multiply -- no fp32 PSUM add needed.
    Masked entries have expb == 0 exactly -> probs match the reference.
  - attention_mask indexes k = partitions -> fused into the Exp
    activation as a per-partition bias.
  - softmax without max-subtraction (scores bounded by ~ +-10).
  - row-sum of exp fused into the AV matmul via a ones column:
      lhsT = [V_h | 1] [128k, 65] -> out rows 0..63 = out^T, row 64 = sum.
  - normalization: recip(sum) broadcast via ones-matmul, DVE multiply.
  - Wo: lhsT = attnT_h [64, 128q], rhs = Wo slice [64, 384] -> natural
    [q, D] partial output, DMA'd out.
"""

import numpy as np

B, Q, D, H, DKV = 2, 2048, 768, 12, 64
HPC = 3              # heads per core
NCORES = 8
NEG = -30000.0       # causal mask addend; exp(x + NEG) == 0.0 in fp32
QC = 512             # q chunk (moving dim)
KT = 128             # k tile (partition dim)
NQC = Q // QC        # 4
NKT = Q // KT        # 16
DC = D // 128        # 6 contraction chunks

_prog_cache = {}


def _build_program():
    import concourse.bass as bass
    import concourse.tile as tile
    from concourse import bacc, mybir
    from concourse.bass import ts

    F32 = mybir.dt.float32
    F16 = mybir.dt.float16
    EXP = mybir.ActivationFunctionType.Exp

    nc = bacc.Bacc("TRN2", target_bir_lowering=False, debug=False)

    xT = nc.dram_tensor("xT", [128, DC, Q], F16, kind="ExternalInput").ap()
    # cols 0:128 = Wq' heads {0,1}; 128:256 = Wk heads {0,1};
    # 256:320 = Wq' head 2; 320:384 = Wk head 2
    wqk = nc.dram_tensor("wqk", [128, DC, 2 * HPC * DKV], F16, kind="ExternalInput").ap()
    wv = nc.dram_tensor("wv", [128, DC, HPC * DKV], F16, kind="ExternalInput").ap()
    wo = nc.dram_tensor("wo", [HPC * DKV, D], F16, kind="ExternalInput").ap()
    expb = nc.dram_tensor("expb", [HPC, NKT, 128, Q], F16, kind="ExternalInput").ap()
    out = nc.dram_tensor("out", [NKT, 128, D], F16, kind="ExternalOutput").ap()

    with tile.TileContext(nc) as tc:
        with (
            nc.allow_low_precision(reason="fp16 matmul operands; fp32 psum accum"),
            tc.tile_pool(name="const", bufs=1) as const,
            tc.tile_pool(name="ps", bufs=2, space="PSUM") as ps,
            tc.tile_pool(name="projps", bufs=1, space="PSUM") as projps,
            tc.tile_pool(name="psO", bufs=2, space="PSUM") as psO,
            tc.tile_pool(name="psF", bufs=1, space="PSUM") as psF,
            tc.tile_pool(name="biasp", bufs=5) as biasp,
            tc.tile_pool(name="expsp", bufs=6) as expsp,
            tc.tile_pool(name="expp", bufs=6) as expp,
            tc.tile_pool(name="small", bufs=3) as small,
            tc.tile_pool(name="outp", bufs=3) as outp,
        ):
            # ---- stage A: load everything ----
            from concourse.tile import add_dep_helper
            wqk_sb = const.tile([128, DC, 2 * HPC * DKV], F16, tag="wqk")
            nc.sync.dma_start(wqk_sb[:], wqk[:])
            wv_sb = const.tile([128, DC, HPC * DKV], F16, tag="wv")
            nc.sync.dma_start(wv_sb[:], wv[:])
            # Wo stacked: [0:128] = heads {0,1} vertically, wo2 = head 2
            wo01_sb = const.tile([2 * DKV, D], F16, tag="wo01")
            nc.sync.dma_start(wo01_sb[:], wo[0:2 * DKV, :])
            wo2_sb = const.tile([DKV, D], F16, tag="wo2")
            nc.sync.dma_start(wo2_sb[:], wo[2 * DKV:, :])
            xT_sb = const.tile([128, DC, Q], F16, tag="xT")
            xT_dmas = [
                nc.sync.dma_start(
                    xT_sb[:, c, ts(hf, Q // 2)], xT[:, c, ts(hf, Q // 2)]
                )
                for c in range(DC)
                for hf in range(2)
            ]
            ones1 = const.tile([1, DKV], F16, tag="ones1")
            nc.gpsimd.memset(ones1[:], 1.0)

            # ---- stage B: projections ----
            # Q^T / K^T storage. Heads 0,1 paired in [128, Q] tiles (head 0 =
            # rows 0:64, head 1 = rows 64:128, so scores-matmul operands share
            # a base partition); head 2 in separate [64, Q] tiles (base 0).
            qT01 = const.tile([128, Q], F16, tag="qT01")
            kT01 = const.tile([128, Q], F16, tag="kT01")
            # head 2: one M=128 group -> qkT2 rows 0:64 = Q^T, 64:128 = K^T;
            # K^T is then DMA-shifted down to kT2b rows 0:64 so the scores
            # matmul operands share base partition 0
            qkT2 = const.tile([128, Q], F16, tag="qkT2")
            kT2b = const.tile([DKV, Q], F16, tag="kT2b")
            # (lhsT weight slice, dest tile) per projection matmul group
            proj_groups = [
                ((0, 128), qT01), ((128, 256), kT01), ((256, 384), qkT2),
            ]
            def make_qk_unit(j, g, pool_tag=None):
                (w0, w1), dst = proj_groups[g]

                def qk_unit():
                    pool, tag = pool_tag or (projps, "pp")
                    p = pool.tile([w1 - w0, QC], F32, tag=tag, name="p")
                    for c in range(DC):
                        nc.tensor.matmul(
                            p[:], wqk_sb[:, c, w0:w1], xT_sb[:, c, ts(j, QC)],
                            start=(c == 0), stop=(c == DC - 1),
                        )
                    nc.scalar.copy(dst[:, ts(j, QC)], p[:])
                    if dst is qkT2:
                        nc.sync.dma_start(
                            kT2b[:, ts(j, QC)], qkT2[DKV:128, ts(j, QC)]
                        )
                return qk_unit

            def qk_slices(h, t, j):
                """(lhsT k-slice, rhs q-slice) for head h, k-tile t, q-chunk j."""
                if h == 0:
                    return kT01[0:DKV, ts(t, KT)], qT01[0:DKV, ts(j, QC)]
                if h == 1:
                    return kT01[DKV:128, ts(t, KT)], qT01[DKV:128, ts(j, QC)]
                return kT2b[:, ts(t, KT)], qkT2[0:DKV, ts(j, QC)]

            # V (natural [k, d]) with a ones column per head: [128, NKT, 3, 65]
            # per (tile t, head h): v1[:, t, h, 0:64] = V_h, v1[:, t, h, 64] = 1
            v1 = const.tile([128, NKT, HPC, DKV + 1], F16, tag="v1")
            nc.gpsimd.memset(v1[:], 1.0)

            def make_v_unit(t, pool_tag=None):
                def v_unit():
                    pool, tag = pool_tag or (projps, "pp")
                    pv = pool.tile([128, HPC * DKV], F32, tag=tag, name="pv")
                    for c in range(DC):
                        nc.tensor.matmul(
                            pv[:], xT_sb[:, c, ts(t, KT)], wv_sb[:, c, :],
                            start=(c == 0), stop=(c == DC - 1),
                        )
                    # single strided copy: [128, 3, 64] <- [128, (3 64)]
                    nc.vector.tensor_copy(
                        v1[:, t, :, 0:DKV],
                        pv[:].rearrange("p (h d) -> p h d", h=HPC),
                    )
                return v_unit

            # ---- stage C: attention (scores^T layout), stage D: Wo ----
            attnT01 = const.tile([2 * DKV, Q], F16, tag="attnT01")
            attnT2 = const.tile([DKV, Q], F16, tag="attnT2")
            def make_norm(po, h, j):
                # normalize: attnT_h[:, jq] = po[0:64] * (1/po[64]) bcast
                def norm():
                    rc = small.tile([1, QC], F16, tag="rc", name="rc")
                    nc.vector.reciprocal(rc[:], po[64:65, :])
                    bc = small.tile([DKV, QC], F16, tag="bc", name="bc")
                    nc.gpsimd.partition_broadcast(bc[:], rc[:])
                    if h == 0:
                        dst = attnT01[0:DKV, ts(j, QC)]
                    elif h == 2:
                        dst = attnT2[:, ts(j, QC)]
                    else:
                        dst = None
                    if dst is not None:
                        nc.vector.tensor_mul(dst, po[0:64, :], bc[:])
                    else:
                        # head 1 lands at partitions 64:128 of the stacked
                        # pair tile; DVE can't shift partitions, so stage at
                        # base 0 and DMA-shift (SBUF->SBUF moves are
                        # partition-agnostic)
                        stg = small.tile([DKV, QC], F16, tag="stg", name="stg")
                        nc.vector.tensor_mul(stg[:], po[0:64, :], bc[:])
                        nc.sync.dma_start(attnT01[DKV:128, ts(j, QC)], stg[:])
                return norm

            def make_wo(i0, tail=False):
                # Wo partial for q-tiles i0, i0+1 (2x128 q rows, one out DMA)
                def wo_i():
                    ot = outp.tile([128, 2, D], F16, tag="ot", name="ot")
                    for ii in range(2):
                        i = i0 + ii
                        for half in range(2):
                            if tail:
                                pool, tag = (ps, "ps") if half else (projps, "pp")
                                pf = pool.tile([128, 384], F32, tag=tag, name="pf")
                            else:
                                pf = psF.tile([128, 384], F32, tag="pf", name="pf")
                            hs = slice(384 * half, 384 * half + 384)
                            nc.tensor.matmul(
                                pf[:], attnT01[:, ts(i, KT)], wo01_sb[:, hs],
                                start=True, stop=False,
                            )
                            nc.tensor.matmul(
                                pf[:], attnT2[:, ts(i, KT)], wo2_sb[:, hs],
                                start=False, stop=True,
                            )
                            if half == 0:
                                nc.vector.tensor_copy(ot[:, ii, 0:384], pf[:])
                            elif i < 8:
                                # early chunks: ACT has slack; late chunks
                                # are ACT-bound, keep copies off its queue
                                nc.scalar.copy(ot[:, ii, 384:768], pf[:])
                            else:
                                nc.vector.tensor_copy(ot[:, ii, 384:768], pf[:])
                    nc.sync.dma_start(
                        out[i0:i0 + 2].rearrange("i p n -> p i n"), ot[:]
                    )
                return wo_i

            # Software pipelining via two drip queues:
            #  - normq: deferred normalization + Wo closures (FIFO keeps
            #    norm(h,j) ahead of wo(j,*) which reads normalized attnT);
            #    popped at (h,j) / tt boundaries so po slots recycle.
            #  - projq: projection units for q-chunk j+1 (Q/K chunk matmuls,
            #    V k-tiles), dripped one per t-step of attention(j) so the
            #    serial projection phase disappears into attention's PE gaps.
            normq = []
            projq = []
            n_bt_dmas = [0]
            # chunk-0 projections must precede attention(0); attention is not
            # running yet, so spread them over the idle pair-pool psum slots
            # to pipeline instead of serializing on the single "pp" slot
            startup_slots = [(projps, "pp"), (ps, "ps"), (psO, "po")]
            for g in range(len(proj_groups)):
                make_qk_unit(0, g, startup_slots[g % 3])()
            for t in range(4):
                make_v_unit(t, startup_slots[t % 3])()

            for j in range(NQC):
                # any leftover proj(j) units must be emitted before
                # attention(j) reads their outputs
                while projq:
                    projq.pop(0)()
                if j + 1 < NQC:
                    for g in range(len(proj_groups)):
                        projq.append(make_qk_unit(j + 1, g))
                    for t in range(4 * (j + 1), 4 * (j + 1) + 4):
                        projq.append(make_v_unit(t))
                # drip projections evenly: 8 units over this j's 12(j+1)
                # t-steps, front-loaded enough to finish before attn(j+1)
                stride = max(1, (6 * (j + 1)) // 9)
                slot = 0
                for h in range(HPC):
                    # free po slots before claiming one (po bufs=2)
                    for _ in range(min(2, len(normq))):
                        normq.pop(0)()
                    po = psO.tile([65, QC], F32, tag="po")
                    nkt = 4 * j + 4  # causal: k-tiles 0..4j+3
                    for tt in range(j + 1):  # batched expb DMA: 4 k-tiles
                        if normq:
                            normq.pop(0)()
                        bt = biasp.tile([128, 4, QC], F16, tag="bt")
                        if tt == j:
                            # diagonal group: second pair only needs the
                            # upper half of the q-chunk
                            bt_dma = nc.sync.dma_start(
                                bt[:, 0:2, :],
                                expb[h, 4 * tt:4 * tt + 2, :, ts(j, QC)]
                                .rearrange("t p q -> p t q"),
                            )
                            nc.sync.dma_start(
                                bt[:, 2:4, 256:],
                                expb[h, 4 * tt + 2:4 * tt + 4, :,
                                     512 * j + 256:512 * j + 512]
                                .rearrange("t p q -> p t q"),
                            )
                        else:
                            bt_dma = nc.sync.dma_start(
                                bt[:],
                                expb[h, 4 * tt:4 * tt + 4, :, ts(j, QC)]
                                .rearrange("t p q -> p t q"),
                            )
                        if n_bt_dmas[0] < 4:
                            # don't let early expb prefetch steal HBM
                            # bandwidth from the critical-path xT load
                            add_dep_helper(
                                bt_dma.ins, xT_dmas[-1].ins,
                                reason="expb prefetch after xT",
                            )
                        n_bt_dmas[0] += 1
                        for pr in range(2):  # two k-tile pairs per tt-group
                            # diagonal narrowing: in the last tt-group the
                            # second pair's tiles (k >= 512j+256) only see
                            # q >= 512j+256, so compute the upper half-chunk
                            # only (the skipped region is causally masked ->
                            # contributes exactly 0)
                            q0 = 256 if (tt == j and pr == 1) else 0
                            pss = ps.tile([128, 2, QC], F32, tag="ps")
                            for half in range(2):
                                t = 4 * tt + 2 * pr + half
                                k_sl, q_sl = qk_slices(h, t, j)
                                nc.tensor.matmul(
                                    pss[:, half, q0:], k_sl, q_sl[:, q0:],
                                    start=True, stop=True,
                                )
                            es = expsp.tile([128, 2, QC], F16, tag="es")
                            nc.scalar.activation(
                                es[:, :, q0:], pss[:, :, q0:], EXP
                            )
                            et = expp.tile([128, 2, QC], F16, tag="et")
                            nc.vector.tensor_mul(
                                et[:, :, q0:], es[:, :, q0:],
                                bt[:, 2 * pr:2 * pr + 2, q0:],
                            )
                            for half in range(2):
                                t = 4 * tt + 2 * pr + half
                                nc.tensor.matmul(
                                    po[:, q0:], v1[:, t, h, :],
                                    et[:, half, q0:],
                                    start=(t == 0), stop=(t == nkt - 1),
                                )
                            if projq and slot % stride == 0:
                                projq.pop(0)()
                            slot += 1
                    normq.append(make_norm(po, h, j))
                for i0 in range(4 * j, 4 * j + 4, 2):
                    normq.append(make_wo(i0, tail=(j == NQC - 1)))
            for fn in normq + projq:
                fn()

    nc.compile()
    return nc


def get_program():
    if "nc" not in _prog_cache:
        _prog_cache["nc"] = _build_program()
    return _prog_cache["nc"]


def make_in_maps(hidden_states, attention_mask, position_bias, Wq, Wk, Wv, Wo):
    hs = np.ascontiguousarray(np.asarray(hidden_states, dtype=np.float32))
    am = np.asarray(attention_mask, dtype=np.float32)
    pb = np.asarray(position_bias, dtype=np.float32)
    wq = np.asarray(Wq, dtype=np.float32) * np.float32(1.0 / np.sqrt(DKV))
    wk = np.asarray(Wk, dtype=np.float32)
    wv_ = np.asarray(Wv, dtype=np.float32)
    wo_ = np.asarray(Wo, dtype=np.float32)

    # causal addend in [k, q] indexing: NEG where k > q
    kk = np.arange(Q, dtype=np.int64)
    causal_T = np.where(kk[:, None] > kk[None, :], np.float32(NEG), np.float32(0.0))
    causal_T = causal_T.astype(np.float32)

    in_maps = []
    for core in range(NCORES):
        b, g = divmod(core, NCORES // B)
        h0 = g * HPC
        # X^T chunked: [128, DC, Q], [p, c, q] = hs[b, q, 128c+p]
        xT = np.ascontiguousarray(
            hs[b].T.reshape(DC, 128, Q).transpose(1, 0, 2)
        ).astype(np.float16)
        # wqk: [128, DC, 384]: [Wq'01 | Wk01 | Wq'2 | Wk2]
        wq_sl = wq[:, h0 * DKV:(h0 + HPC) * DKV]
        wk_sl = wk[:, h0 * DKV:(h0 + HPC) * DKV]
        wqk = np.concatenate([
            wq_sl[:, 0:128], wk_sl[:, 0:128],
            wq_sl[:, 128:192], wk_sl[:, 128:192],
        ], axis=1)  # (D, 384)
        wqk = np.ascontiguousarray(
            wqk.reshape(DC, 128, 2 * HPC * DKV).transpose(1, 0, 2)
        ).astype(np.float16)
        # wv: [128, DC, HPC*DKV]
        wv_sl = wv_[:, (h0) * DKV:(h0 + HPC) * DKV].reshape(DC, 128, HPC * DKV)
        wv_sl = np.ascontiguousarray(wv_sl.transpose(1, 0, 2)).astype(np.float16)
        # wo: [DKV, HPC, D]: [p, h, n] = Wo[(h0+h)*DKV + p, n]
        wo_sl = np.ascontiguousarray(
            wo_[h0 * DKV:(h0 + HPC) * DKV, :]
        ).astype(np.float16)
        # expb: [HPC, NKT, 128, Q]: exp(biasT + causal + mask_k); masked -> 0
        # (attention_mask indexes k, which is the row dim of the transposed
        # bias, so it folds in as a per-row addend before the exp)
        bT = pb[0, h0:h0 + HPC].transpose(0, 2, 1) + causal_T[None]
        bT += am[b, 0, 0][None, :, None]
        bT = np.exp(bT, out=bT)
        bT = np.ascontiguousarray(bT.reshape(HPC, NKT, 128, Q)).astype(np.float16)
        in_maps.append({
            "xT": xT, "wqk": wqk, "wv": wv_sl, "wo": wo_sl,
            "expb": bT,
        })
    return in_maps


def kernel(hidden_states, attention_mask, position_bias, Wq, Wk, Wv, Wo):
    from concourse.bass_utils import run_bass_kernel_spmd

    nc = get_program()
    in_maps = make_in_maps(
        hidden_states, attention_mask, position_bias, Wq, Wk, Wv, Wo
    )
    res = run_bass_kernel_spmd(nc, in_maps, list(range(NCORES)))
    out = np.zeros((B, Q, D), dtype=np.float32)
    for core in range(NCORES):
        b = core // (NCORES // B)
        out[b] += res.results[core]["out"].reshape(Q, D).astype(np.float32)
    return out


# revision 81
# speedup vs baseline: 1.1529x; 1.0137x over previous
"""Bass/Tile TRN2 kernel for nn_MultiHeadAttention_4329327034628.

Multi-head self-attention with additive position bias + causal mask
(T5-style), B=2, Q=2048, D=768, H=12, DKV=64, fp32.

Sharding over 8 NeuronCores: core k -> (batch b = k//4, head-group
g = k%4 of 3 heads).  Each core computes its heads' attention and a
partial output projection (attn @ Wo_slice); the host sums the 4
partials per batch (the post-Wo all-reduce done at gather time).

Device-side layout strategy (no on-chip transposes needed):
  - host ships X^T (D on partitions) -> QKV projections contract D.
  - Q^T, K^T kept as [dkv, q]; scores computed transposed:
      scores^T [k, q] = lhsT(K^T slice).T @ rhs(Q^T)   (contract dkv)
  - position_bias is pre-transposed on host to [k, q] tiles, the causal
    NEG added, and *exponentiated* (expb = exp(biasT + causal), fp16):
    exp(s + b) = exp(s) * exp(b), so the device does ACT exp(s) followed
    by a cheap fp16 2x-mode DVE multiply -- no fp32 PSUM add needed.
    Masked entries have expb == 0 exactly -> probs match the reference.
  - attention_mask indexes k = partitions -> fused into the Exp
    activation as a per-partition bias.
  - softmax without max-subtraction (scores bounded by ~ +-10).
  - row-sum of exp fused into the AV matmul via a ones column:
      lhsT = [V_h | 1] [128k, 65] -> out rows 0..63 = out^T, row 64 = sum.
  - normalization: recip(sum) broadcast via ones-matmul, DVE multiply.
  - Wo: lhsT = attnT_h [64, 128q], rhs = Wo slice [64, 384] -> natural
    [q, D] partial output, DMA'd out.
"""

import numpy as np

B, Q, D, H, DKV = 2, 2048, 768, 12, 64
HPC = 3              # heads per core
NCORES = 8
NEG = -30000.0       # causal mask addend; exp(x + NEG) == 0.0 in fp32
QC = 512             # q chunk (moving dim)
KT = 128             # k tile (partition dim)
NQC = Q // QC        # 4
NKT = Q // KT        # 16
DC = D // 128        # 6 contraction chunks

_prog_cache = {}


def _build_program():
    import concourse.bass as bass
    import concourse.tile as tile
    from concourse import bacc, mybir
    from concourse.bass import ts

    F32 = mybir.dt.float32
    F16 = mybir.dt.float16
    EXP = mybir.ActivationFunctionType.Exp

    nc = bacc.Bacc("TRN2", target_bir_lowering=False, debug=False)

    xT = nc.dram_tensor("xT", [128, DC, Q], F16, kind="ExternalInput").ap()
    # cols 0:128 = Wq' heads {0,1}; 128:256 = Wk heads {0,1};
    # 256:320 = Wq' head 2; 320:384 = Wk head 2
    wqk = nc.dram_tensor("wqk", [128, DC, 2 * HPC * DKV], F16, kind="ExternalInput").ap()
    wv = nc.dram_tensor("wv", [128, DC, HPC * DKV], F16, kind="ExternalInput").ap()
    wo = nc.dram_tensor("wo", [HPC * DKV, D], F16, kind="ExternalInput").ap()
    expb = nc.dram_tensor("expb", [HPC, NKT, 128, Q], F16, kind="ExternalInput").ap()
    out = nc.dram_tensor("out", [NKT, 128, D], F16, kind="ExternalOutput").ap()

    with tile.TileContext(nc) as tc:
        with (
            nc.allow_low_precision(reason="fp16 matmul operands; fp32 psum accum"),
            tc.tile_pool(name="const", bufs=1) as const,
            tc.tile_pool(name="ps", bufs=2, space="PSUM") as ps,
            tc.tile_pool(name="projps", bufs=1, space="PSUM") as projps,
            tc.tile_pool(name="psO", bufs=2, space="PSUM") as psO,
            tc.tile_pool(name="psF", bufs=1, space="PSUM") as psF,
            tc.tile_pool(name="biasp", bufs=5) as biasp,
            tc.tile_pool(name="expsp", bufs=6) as expsp,
            tc.tile_pool(name="expp", bufs=6) as expp,
            tc.tile_pool(name="small", bufs=3) as small,
            tc.tile_pool(name="outp", bufs=3) as outp,
        ):
            # ---- stage A: load everything ----
            from concourse.tile import add_dep_helper
            wqk_sb = const.tile([128, DC, 2 * HPC * DKV], F16, tag="wqk")
            nc.sync.dma_start(wqk_sb[:], wqk[:])
            wv_sb = const.tile([128, DC, HPC * DKV], F16, tag="wv")
            nc.sync.dma_start(wv_sb[:], wv[:])
            # Wo stacked: [0:128] = heads {0,1} vertically, wo2 = head 2
            wo01_sb = const.tile([2 * DKV, D], F16, tag="wo01")
            nc.sync.dma_start(wo01_sb[:], wo[0:2 * DKV, :])
            wo2_sb = const.tile([DKV, D], F16, tag="wo2")
            nc.sync.dma_start(wo2_sb[:], wo[2 * DKV:, :])
            xT_sb = const.tile([128, DC, Q], F16, tag="xT")
            # first-half q-columns land first: chunk-0/1 projections only
            # read those, so the PE unblocks ~4us sooner
            xT_dmas = [
                nc.sync.dma_start(
                    xT_sb[:, c, ts(hf, Q // 2)], xT[:, c, ts(hf, Q // 2)]
                )
                for hf in range(2)
                for c in range(DC)
            ]
            ones1 = const.tile([1, DKV], F16, tag="ones1")
            nc.gpsimd.memset(ones1[:], 1.0)

            # ---- stage B: projections ----
            # Q^T / K^T storage. Heads 0,1 paired in [128, Q] tiles (head 0 =
            # rows 0:64, head 1 = rows 64:128, so scores-matmul operands share
            # a base partition); head 2 in separate [64, Q] tiles (base 0).
            qT01 = const.tile([128, Q], F16, tag="qT01")
            kT01 = const.tile([128, Q], F16, tag="kT01")
            # head 2: one M=128 group -> qkT2 rows 0:64 = Q^T, 64:128 = K^T;
            # K^T is then DMA-shifted down to kT2b rows 0:64 so the scores
            # matmul operands share base partition 0
            qkT2 = const.tile([128, Q], F16, tag="qkT2")
            kT2b = const.tile([DKV, Q], F16, tag="kT2b")
            # (lhsT weight slice, dest tile) per projection matmul group
            proj_groups = [
                ((0, 128), qT01), ((128, 256), kT01), ((256, 384), qkT2),
            ]
            def make_qk_unit(j, g, pool_tag=None):
                (w0, w1), dst = proj_groups[g]

                def qk_unit():
                    pool, tag = pool_tag or (projps, "pp")
                    p = pool.tile([w1 - w0, QC], F32, tag=tag, name="p")
                    for c in range(DC):
                        nc.tensor.matmul(
                            p[:], wqk_sb[:, c, w0:w1], xT_sb[:, c, ts(j, QC)],
                            start=(c == 0), stop=(c == DC - 1),
                        )
                    nc.scalar.copy(dst[:, ts(j, QC)], p[:])
                    if dst is qkT2:
                        nc.sync.dma_start(
                            kT2b[:, ts(j, QC)], qkT2[DKV:128, ts(j, QC)]
                        )
                return qk_unit

            def qk_slices(h, t, j):
                """(lhsT k-slice, rhs q-slice) for head h, k-tile t, q-chunk j."""
                if h == 0:
                    return kT01[0:DKV, ts(t, KT)], qT01[0:DKV, ts(j, QC)]
                if h == 1:
                    return kT01[DKV:128, ts(t, KT)], qT01[DKV:128, ts(j, QC)]
                return kT2b[:, ts(t, KT)], qkT2[0:DKV, ts(j, QC)]

            # V (natural [k, d]) with a ones column per head: [128, NKT, 3, 65]
            # per (tile t, head h): v1[:, t, h, 0:64] = V_h, v1[:, t, h, 64] = 1
            v1 = const.tile([128, NKT, HPC, DKV + 1], F16, tag="v1")
            nc.gpsimd.memset(v1[:], 1.0)

            def make_v_unit(t, pool_tag=None):
                def v_unit():
                    pool, tag = pool_tag or (projps, "pp")
                    pv = pool.tile([128, HPC * DKV], F32, tag=tag, name="pv")
                    for c in range(DC):
                        nc.tensor.matmul(
                            pv[:], xT_sb[:, c, ts(t, KT)], wv_sb[:, c, :],
                            start=(c == 0), stop=(c == DC - 1),
                        )
                    # single strided copy: [128, 3, 64] <- [128, (3 64)]
                    nc.vector.tensor_copy(
                        v1[:, t, :, 0:DKV],
                        pv[:].rearrange("p (h d) -> p h d", h=HPC),
                    )
                return v_unit

            # ---- stage C: attention (scores^T layout), stage D: Wo ----
            attnT01 = const.tile([2 * DKV, Q], F16, tag="attnT01")
            attnT2 = const.tile([DKV, Q], F16, tag="attnT2")
            def make_norm(po, h, j):
                # normalize: attnT_h[:, jq] = po[0:64] * (1/po[64]) bcast
                def norm():
                    rc = small.tile([1, QC], F16, tag="rc", name="rc")
                    nc.vector.reciprocal(rc[:], po[64:65, :])
                    bc = small.tile([DKV, QC], F16, tag="bc", name="bc")
                    nc.gpsimd.partition_broadcast(bc[:], rc[:])
                    if h == 0:
                        dst = attnT01[0:DKV, ts(j, QC)]
                    elif h == 2:
                        dst = attnT2[:, ts(j, QC)]
                    else:
                        dst = None
                    if dst is not None:
                        nc.vector.tensor_mul(dst, po[0:64, :], bc[:])
                    else:
                        # head 1 lands at partitions 64:128 of the stacked
                        # pair tile; DVE can't shift partitions, so stage at
                        # base 0 and DMA-shift (SBUF->SBUF moves are
                        # partition-agnostic)
                        stg = small.tile([DKV, QC], F16, tag="stg", name="stg")
                        nc.vector.tensor_mul(stg[:], po[0:64, :], bc[:])
                        nc.sync.dma_start(attnT01[DKV:128, ts(j, QC)], stg[:])
                return norm

            def make_wo(i0, tail=False):
                # Wo partial for q-tiles i0, i0+1 (2x128 q rows, one out DMA)
                def wo_i():
                    ot = outp.tile([128, 2, D], F16, tag="ot", name="ot")
                    for ii in range(2):
                        i = i0 + ii
                        for half in range(2):
                            if tail:
                                pool, tag = (ps, "ps") if half else (projps, "pp")
                                pf = pool.tile([128, 384], F32, tag=tag, name="pf")
                            else:
                                pf = psF.tile([128, 384], F32, tag="pf", name="pf")
                            hs = slice(384 * half, 384 * half + 384)
                            nc.tensor.matmul(
                                pf[:], attnT01[:, ts(i, KT)], wo01_sb[:, hs],
                                start=True, stop=False,
                            )
                            nc.tensor.matmul(
                                pf[:], attnT2[:, ts(i, KT)], wo2_sb[:, hs],
                                start=False, stop=True,
                            )
                            if half == 0:
                                nc.vector.tensor_copy(ot[:, ii, 0:384], pf[:])
                            elif i < 8:
                                # early chunks: ACT has slack; late chunks
                                # are ACT-bound, keep copies off its queue
                                nc.scalar.copy(ot[:, ii, 384:768], pf[:])
                            else:
                                nc.vector.tensor_copy(ot[:, ii, 384:768], pf[:])
                    nc.sync.dma_start(
                        out[i0:i0 + 2].rearrange("i p n -> p i n"), ot[:]
                    )
                return wo_i

            # Software pipelining via two drip queues:
            #  - normq: deferred normalization + Wo closures (FIFO keeps
            #    norm(h,j) ahead of wo(j,*) which reads normalized attnT);
            #    popped at (h,j) / tt boundaries so po slots recycle.
            #  - projq: projection units for q-chunk j+1 (Q/K chunk matmuls,
            #    V k-tiles), dripped one per t-step of attention(j) so the
            #    serial projection phase disappears into attention's PE gaps.
            normq = []
            projq = []
            n_bt_dmas = [0]
            # chunk-0 projections must precede attention(0); attention is not
            # running yet, so spread them over the idle pair-pool psum slots
            # to pipeline instead of serializing on the single "pp" slot
            startup_slots = [(projps, "pp"), (ps, "ps"), (psO, "po")]
            for g in range(len(proj_groups)):
                make_qk_unit(0, g, startup_slots[g % 3])()
            for t in range(4):
                make_v_unit(t, startup_slots[t % 3])()

            for j in range(NQC):
                # any leftover proj(j) units must be emitted before
                # attention(j) reads their outputs
                while projq:
                    projq.pop(0)()
                if j + 1 < NQC:
                    for g in range(len(proj_groups)):
                        projq.append(make_qk_unit(j + 1, g))
                    for t in range(4 * (j + 1), 4 * (j + 1) + 4):
                        projq.append(make_v_unit(t))
                # drip projections evenly: 8 units over this j's 12(j+1)
                # t-steps, front-loaded enough to finish before attn(j+1)
                stride = max(1, (6 * (j + 1)) // 9)
                slot = 0
                for h in range(HPC):
                    # free po slots before claiming one (po bufs=2)
                    for _ in range(min(2, len(normq))):
                        normq.pop(0)()
                    po = psO.tile([65, QC], F32, tag="po")
                    nkt = 4 * j + 4  # causal: k-tiles 0..4j+3
                    for tt in range(j + 1):  # batched expb DMA: 4 k-tiles
                        if normq:
                            normq.pop(0)()
                        bt = biasp.tile([128, 4, QC], F16, tag="bt")
                        if tt == j:
                            # diagonal group: second pair only needs the
                            # upper half of the q-chunk
                            bt_dma = nc.sync.dma_start(
                                bt[:, 0:2, :],
                                expb[h, 4 * tt:4 * tt + 2, :, ts(j, QC)]
                                .rearrange("t p q -> p t q"),
                            )
                            nc.sync.dma_start(
                                bt[:, 2:4, 256:],
                                expb[h, 4 * tt + 2:4 * tt + 4, :,
                                     512 * j + 256:512 * j + 512]
                                .rearrange("t p q -> p t q"),
                            )
                        else:
                            bt_dma = nc.sync.dma_start(
                                bt[:],
                                expb[h, 4 * tt:4 * tt + 4, :, ts(j, QC)]
                                .rearrange("t p q -> p t q"),
                            )
                        if n_bt_dmas[0] < 4:
                            # don't let early expb prefetch steal HBM
                            # bandwidth from the critical-path xT load
                            add_dep_helper(
                                bt_dma.ins, xT_dmas[-1].ins,
                                reason="expb prefetch after xT",
                            )
                        n_bt_dmas[0] += 1
                        for pr in range(2):  # two k-tile pairs per tt-group
                            # diagonal narrowing: in the last tt-group the
                            # second pair's tiles (k >= 512j+256) only see
                            # q >= 512j+256, so compute the upper half-chunk
                            # only (the skipped region is causally masked ->
                            # contributes exactly 0)
                            q0 = 256 if (tt == j and pr == 1) else 0
                            pss = ps.tile([128, 2, QC], F32, tag="ps")
                            for half in range(2):
                                t = 4 * tt + 2 * pr + half
                                k_sl, q_sl = qk_slices(h, t, j)
                                nc.tensor.matmul(
                                    pss[:, half, q0:], k_sl, q_sl[:, q0:],
                                    start=True, stop=True,
                                )
                            es = expsp.tile([128, 2, QC], F16, tag="es")
                            nc.scalar.activation(
                                es[:, :, q0:], pss[:, :, q0:], EXP
                            )
                            et = expp.tile([128, 2, QC], F16, tag="et")
                            nc.vector.tensor_mul(
                                et[:, :, q0:], es[:, :, q0:],
                                bt[:, 2 * pr:2 * pr + 2, q0:],
                            )
                            for half in range(2):
                                t = 4 * tt + 2 * pr + half
                                nc.tensor.matmul(
                                    po[:, q0:], v1[:, t, h, :],
                                    et[:, half, q0:],
                                    start=(t == 0), stop=(t == nkt - 1),
                                )
                            if projq and slot % stride == 0:
                                projq.pop(0)()
                            slot += 1
                    normq.append(make_norm(po, h, j))
                for i0 in range(4 * j, 4 * j + 4, 2):
                    normq.append(make_wo(i0, tail=(j == NQC - 1)))
            for fn in normq + projq:
                fn()

    nc.compile()
    return nc


def get_program():
    if "nc" not in _prog_cache:
        _prog_cache["nc"] = _build_program()
    return _prog_cache["nc"]


def make_in_maps(hidden_states, attention_mask, position_bias, Wq, Wk, Wv, Wo):
    hs = np.ascontiguousarray(np.asarray(hidden_states, dtype=np.float32))
    am = np.asarray(attention_mask, dtype=np.float32)
    pb = np.asarray(position_bias, dtype=np.float32)
    wq = np.asarray(Wq, dtype=np.float32) * np.float32(1.0 / np.sqrt(DKV))
    wk = np.asarray(Wk, dtype=np.float32)
    wv_ = np.asarray(Wv, dtype=np.float32)
    wo_ = np.asarray(Wo, dtype=np.float32)

    # causal addend in [k, q] indexing: NEG where k > q
    kk = np.arange(Q, dtype=np.int64)
    causal_T = np.where(kk[:, None] > kk[None, :], np.float32(NEG), np.float32(0.0))
    causal_T = causal_T.astype(np.float32)

    in_maps = []
    for core in range(NCORES):
        b, g = divmod(core, NCORES // B)
        h0 = g * HPC
        # X^T chunked: [128, DC, Q], [p, c, q] = hs[b, q, 128c+p]
        xT = np.ascontiguousarray(
            hs[b].T.reshape(DC, 128, Q).transpose(1, 0, 2)
        ).astype(np.float16)
        # wqk: [128, DC, 384]: [Wq'01 | Wk01 | Wq'2 | Wk2]
        wq_sl = wq[:, h0 * DKV:(h0 + HPC) * DKV]
        wk_sl = wk[:, h0 * DKV:(h0 + HPC) * DKV]
        wqk = np.concatenate([
            wq_sl[:, 0:128], wk_sl[:, 0:128],
            wq_sl[:, 128:192], wk_sl[:, 128:192],
        ], axis=1)  # (D, 384)
        wqk = np.ascontiguousarray(
            wqk.reshape(DC, 128, 2 * HPC * DKV).transpose(1, 0, 2)
        ).astype(np.float16)
        # wv: [128, DC, HPC*DKV]
        wv_sl = wv_[:, (h0) * DKV:(h0 + HPC) * DKV].reshape(DC, 128, HPC * DKV)
        wv_sl = np.ascontiguousarray(wv_sl.transpose(1, 0, 2)).astype(np.float16)
        # wo: [DKV, HPC, D]: [p, h, n] = Wo[(h0+h)*DKV + p, n]
        wo_sl = np.ascontiguousarray(
            wo_[h0 * DKV:(h0 + HPC) * DKV, :]
        ).astype(np.float16)
        # expb: [HPC, NKT, 128, Q]: exp(biasT + causal + mask_k); masked -> 0
        # (attention_mask indexes k, which is the row dim of the transposed
        # bias, so it folds in as a per-row addend before the exp)
        bT = pb[0, h0:h0 + HPC].transpose(0, 2, 1) + causal_T[None]
        bT += am[b, 0, 0][None, :, None]
        bT = np.exp(bT, out=bT)
        bT = np.ascontiguousarray(bT.reshape(HPC, NKT, 128, Q)).astype(np.float16)
        in_maps.append({
            "xT": xT, "wqk": wqk, "wv": wv_sl, "wo": wo_sl,
            "expb": bT,
        })
    return in_maps


def kernel(hidden_states, attention_mask, position_bias, Wq, Wk, Wv, Wo):
    from concourse.bass_utils import run_bass_kernel_spmd

    nc = get_program()
    in_maps = make_in_maps(
        hidden_states, attention_mask, position_bias, Wq, Wk, Wv, Wo
    )
    res = run_bass_kernel_spmd(nc, in_maps, list(range(NCORES)))
    out = np.zeros((B, Q, D), dtype=np.float32)
    for core in range(NCORES):
        b = core // (NCORES // B)
        out[b] += res.results[core]["out"].reshape(Q, D).astype(np.float32)
    return out


# revision 82
# speedup vs baseline: 1.1570x; 1.0036x over previous
"""Bass/Tile TRN2 kernel for nn_MultiHeadAttention_4329327034628.

Multi-head self-attention with additive position bias + causal mask
(T5-style), B=2, Q=2048, D=768, H=12, DKV=64, fp32.

Sharding over 8 NeuronCores: core k -> (batch b = k//4, head-group
g = k%4 of 3 heads).  Each core computes its heads' attention and a
partial output projection (attn @ Wo_slice); the host sums the 4
partials per batch (the post-Wo all-reduce done at gather time).

Device-side layout strategy (no on-chip transposes needed):
  - host ships X^T (D on partitions) -> QKV projections contract D.
  - Q^T, K^T kept as [dkv, q]; scores computed transposed:
      scores^T [k, q] = lhsT(K^T slice).T @ rhs(Q^T)   (contract dkv)
  - position_bias is pre-transposed on host to [k, q] tiles, the causal
    NEG added, and *exponentiated* (expb = exp(biasT + causal), fp16):
    exp(s + b) = exp(s) * exp(b), so the device does ACT exp(s) followed
    by a cheap fp16 2x-mode DVE multiply -- no fp32 PSUM add needed.
    Masked entries have expb == 0 exactly -> probs match the reference.
  - attention_mask indexes k = partitions -> fused into the Exp
    activation as a per-partition bias.
  - softmax without max-subtraction (scores bounded by ~ +-10).
  - row-sum of exp fused into the AV matmul via a ones column:
      lhsT = [V_h | 1] [128k, 65] -> out rows 0..63 = out^T, row 64 = sum.
  - normalization: recip(sum) broadcast via ones-matmul, DVE multiply.
  - Wo: lhsT = attnT_h [64, 128q], rhs = Wo slice [64, 384] -> natural
    [q, D] partial output, DMA'd out.
"""

import numpy as np

B, Q, D, H, DKV = 2, 2048, 768, 12, 64
HPC = 3              # heads per core
NCORES = 8
NEG = -30000.0       # causal mask addend; exp(x + NEG) == 0.0 in fp32
QC = 512             # q chunk (moving dim)
KT = 128             # k tile (partition dim)
NQC = Q // QC        # 4
NKT = Q // KT        # 16
DC = D // 128        # 6 contraction chunks

_prog_cache = {}


def _build_program():
    import concourse.bass as bass
    import concourse.tile as tile
    from concourse import bacc, mybir
    from concourse.bass import ts

    F32 = mybir.dt.float32
    F16 = mybir.dt.float16
    EXP = mybir.ActivationFunctionType.Exp

    nc = bacc.Bacc("TRN2", target_bir_lowering=False, debug=False)

    xT = nc.dram_tensor("xT", [128, DC, Q], F16, kind="ExternalInput").ap()
    # cols 0:128 = Wq' heads {0,1}; 128:256 = Wk heads {0,1};
    # 256:320 = Wq' head 2; 320:384 = Wk head 2
    wqk = nc.dram_tensor("wqk", [128, DC, 2 * HPC * DKV], F16, kind="ExternalInput").ap()
    wv = nc.dram_tensor("wv", [128, DC, HPC * DKV], F16, kind="ExternalInput").ap()
    wo = nc.dram_tensor("wo", [HPC * DKV, D], F16, kind="ExternalInput").ap()
    expb = nc.dram_tensor("expb", [HPC, NKT, 128, Q], F16, kind="ExternalInput").ap()
    out = nc.dram_tensor("out", [NKT, 128, D], F16, kind="ExternalOutput").ap()

    with tile.TileContext(nc) as tc:
        with (
            nc.allow_low_precision(reason="fp16 matmul operands; fp32 psum accum"),
            tc.tile_pool(name="const", bufs=1) as const,
            tc.tile_pool(name="ps", bufs=2, space="PSUM") as ps,
            tc.tile_pool(name="projps", bufs=1, space="PSUM") as projps,
            tc.tile_pool(name="psO", bufs=2, space="PSUM") as psO,
            tc.tile_pool(name="psF", bufs=1, space="PSUM") as psF,
            tc.tile_pool(name="biasp", bufs=5) as biasp,
            tc.tile_pool(name="expsp", bufs=6) as expsp,
            tc.tile_pool(name="expp", bufs=6) as expp,
            tc.tile_pool(name="small", bufs=3) as small,
            tc.tile_pool(name="outp", bufs=3) as outp,
        ):
            # ---- stage A: load everything ----
            from concourse.tile import add_dep_helper
            wqk_sb = const.tile([128, DC, 2 * HPC * DKV], F16, tag="wqk")
            nc.sync.dma_start(wqk_sb[:], wqk[:])
            wv_sb = const.tile([128, DC, HPC * DKV], F16, tag="wv")
            nc.sync.dma_start(wv_sb[:], wv[:])
            # Wo stacked: [0:128] = heads {0,1} vertically, wo2 = head 2
            wo01_sb = const.tile([2 * DKV, D], F16, tag="wo01")
            nc.sync.dma_start(wo01_sb[:], wo[0:2 * DKV, :])
            wo2_sb = const.tile([DKV, D], F16, tag="wo2")
            nc.sync.dma_start(wo2_sb[:], wo[2 * DKV:, :])
            xT_sb = const.tile([128, DC, Q], F16, tag="xT")
            # first-half q-columns land first: chunk-0/1 projections only
            # read those, so the PE unblocks ~4us sooner
            xT_dmas = [
                nc.sync.dma_start(
                    xT_sb[:, c, ts(hf, Q // 2)], xT[:, c, ts(hf, Q // 2)]
                )
                for hf in range(2)
                for c in range(DC)
            ]
            ones1 = const.tile([1, DKV], F16, tag="ones1")
            nc.gpsimd.memset(ones1[:], 1.0)

            # ---- stage B: projections ----
            # Q^T / K^T storage. Heads 0,1 paired in [128, Q] tiles (head 0 =
            # rows 0:64, head 1 = rows 64:128, so scores-matmul operands share
            # a base partition); head 2 in separate [64, Q] tiles (base 0).
            qT01 = const.tile([128, Q], F16, tag="qT01")
            kT01 = const.tile([128, Q], F16, tag="kT01")
            # head 2: one M=128 group -> qkT2 rows 0:64 = Q^T, 64:128 = K^T;
            # K^T is then DMA-shifted down to kT2b rows 0:64 so the scores
            # matmul operands share base partition 0
            qkT2 = const.tile([128, Q], F16, tag="qkT2")
            kT2b = const.tile([DKV, Q], F16, tag="kT2b")
            # (lhsT weight slice, dest tile) per projection matmul group
            proj_groups = [
                ((0, 128), qT01), ((128, 256), kT01), ((256, 384), qkT2),
            ]
            def make_qk_unit(j, g, pool_tag=None):
                (w0, w1), dst = proj_groups[g]

                def qk_unit():
                    pool, tag = pool_tag or (projps, "pp")
                    p = pool.tile([w1 - w0, QC], F32, tag=tag, name="p")
                    for c in range(DC):
                        nc.tensor.matmul(
                            p[:], wqk_sb[:, c, w0:w1], xT_sb[:, c, ts(j, QC)],
                            start=(c == 0), stop=(c == DC - 1),
                        )
                    nc.scalar.copy(dst[:, ts(j, QC)], p[:])
                    if dst is qkT2:
                        nc.sync.dma_start(
                            kT2b[:, ts(j, QC)], qkT2[DKV:128, ts(j, QC)]
                        )
                return qk_unit

            def qk_slices(h, t, j):
                """(lhsT k-slice, rhs q-slice) for head h, k-tile t, q-chunk j."""
                if h == 0:
                    return kT01[0:DKV, ts(t, KT)], qT01[0:DKV, ts(j, QC)]
                if h == 1:
                    return kT01[DKV:128, ts(t, KT)], qT01[DKV:128, ts(j, QC)]
                return kT2b[:, ts(t, KT)], qkT2[0:DKV, ts(j, QC)]

            # V (natural [k, d]) with a ones column per head: [128, NKT, 3, 65]
            # per (tile t, head h): v1[:, t, h, 0:64] = V_h, v1[:, t, h, 64] = 1
            v1 = const.tile([128, NKT, HPC, DKV + 1], F16, tag="v1")
            nc.gpsimd.memset(v1[:], 1.0)

            def make_v_unit(t, pool_tag=None):
                def v_unit():
                    pool, tag = pool_tag or (projps, "pp")
                    pv = pool.tile([128, HPC * DKV], F32, tag=tag, name="pv")
                    for c in range(DC):
                        nc.tensor.matmul(
                            pv[:], xT_sb[:, c, ts(t, KT)], wv_sb[:, c, :],
                            start=(c == 0), stop=(c == DC - 1),
                        )
                    # single strided copy: [128, 3, 64] <- [128, (3 64)]
                    nc.vector.tensor_copy(
                        v1[:, t, :, 0:DKV],
                        pv[:].rearrange("p (h d) -> p h d", h=HPC),
                    )
                return v_unit

            # ---- stage C: attention (scores^T layout), stage D: Wo ----
            attnT01 = const.tile([2 * DKV, Q], F16, tag="attnT01")
            attnT2 = const.tile([DKV, Q], F16, tag="attnT2")
            def make_norm(po, h, j):
                # normalize: attnT_h[:, jq] = po[0:64] * (1/po[64]) bcast
                def norm():
                    rc = small.tile([1, QC], F16, tag="rc", name="rc")
                    nc.vector.reciprocal(rc[:], po[64:65, :])
                    bc = small.tile([DKV, QC], F16, tag="bc", name="bc")
                    nc.gpsimd.partition_broadcast(bc[:], rc[:])
                    if h == 0:
                        dst = attnT01[0:DKV, ts(j, QC)]
                    elif h == 2:
                        dst = attnT2[:, ts(j, QC)]
                    else:
                        dst = None
                    if dst is not None:
                        nc.vector.tensor_mul(dst, po[0:64, :], bc[:])
                    else:
                        # head 1 lands at partitions 64:128 of the stacked
                        # pair tile; DVE can't shift partitions, so stage at
                        # base 0 and DMA-shift (SBUF->SBUF moves are
                        # partition-agnostic)
                        stg = small.tile([DKV, QC], F16, tag="stg", name="stg")
                        nc.vector.tensor_mul(stg[:], po[0:64, :], bc[:])
                        nc.sync.dma_start(attnT01[DKV:128, ts(j, QC)], stg[:])
                return norm

            def make_wo(i0, tail=False):
                # Wo partial for q-tiles i0, i0+1 (2x128 q rows, one out DMA)
                def wo_i():
                    ot = outp.tile([128, 2, D], F16, tag="ot", name="ot")
                    for ii in range(2):
                        i = i0 + ii
                        for half in range(2):
                            if tail:
                                pool, tag = (ps, "ps") if half else (projps, "pp")
                                pf = pool.tile([128, 384], F32, tag=tag, name="pf")
                            else:
                                pf = psF.tile([128, 384], F32, tag="pf", name="pf")
                            hs = slice(384 * half, 384 * half + 384)
                            nc.tensor.matmul(
                                pf[:], attnT01[:, ts(i, KT)], wo01_sb[:, hs],
                                start=True, stop=False,
                            )
                            nc.tensor.matmul(
                                pf[:], attnT2[:, ts(i, KT)], wo2_sb[:, hs],
                                start=False, stop=True,
                            )
                            if half == 0:
                                nc.vector.tensor_copy(ot[:, ii, 0:384], pf[:])
                            elif i < 8:
                                # early chunks: ACT has slack; late chunks
                                # are ACT-bound, keep copies off its queue
                                nc.scalar.copy(ot[:, ii, 384:768], pf[:])
                            else:
                                nc.vector.tensor_copy(ot[:, ii, 384:768], pf[:])
                    nc.sync.dma_start(
                        out[i0:i0 + 2].rearrange("i p n -> p i n"), ot[:]
                    )
                return wo_i

            # Software pipelining via two drip queues:
            #  - normq: deferred normalization + Wo closures (FIFO keeps
            #    norm(h,j) ahead of wo(j,*) which reads normalized attnT);
            #    popped at (h,j) / tt boundaries so po slots recycle.
            #  - projq: projection units for q-chunk j+1 (Q/K chunk matmuls,
            #    V k-tiles), dripped one per t-step of attention(j) so the
            #    serial projection phase disappears into attention's PE gaps.
            normq = []
            projq = []
            n_bt_dmas = [0]
            # chunk-0 projections must precede attention(0); attention is not
            # running yet, so spread them over the idle pair-pool psum slots
            # to pipeline instead of serializing on the single "pp" slot
            startup_slots = [(projps, "pp"), (ps, "ps"), (psO, "po")]
            for g in range(len(proj_groups)):
                make_qk_unit(0, g, startup_slots[g % 3])()
            for t in range(4):
                make_v_unit(t, startup_slots[t % 3])()

            for j in range(NQC):
                # any leftover proj(j) units must be emitted before
                # attention(j) reads their outputs
                while projq:
                    projq.pop(0)()
                if j + 1 < NQC:
                    for g in range(len(proj_groups)):
                        projq.append(make_qk_unit(j + 1, g))
                    for t in range(4 * (j + 1), 4 * (j + 1) + 4):
                        projq.append(make_v_unit(t))
                # drip projections evenly: 8 units over this j's 12(j+1)
                # t-steps, front-loaded enough to finish before attn(j+1)
                stride = max(1, (6 * (j + 1)) // 9)
                slot = 0
                for h in range(HPC):
                    # free po slots before claiming one (po bufs=2)
                    for _ in range(min(2, len(normq))):
                        normq.pop(0)()
                    po = psO.tile([65, QC], F32, tag="po")
                    nkt = 4 * j + 4  # causal: k-tiles 0..4j+3
                    for tt in range(j + 1):  # batched expb DMA: 4 k-tiles
                        if normq:
                            normq.pop(0)()
                        bt = biasp.tile([128, 4, QC], F16, tag="bt")
                        if tt == j:
                            # diagonal group: second pair only needs the
                            # upper half of the q-chunk
                            bt_dma = nc.sync.dma_start(
                                bt[:, 0:2, :],
                                expb[h, 4 * tt:4 * tt + 2, :, ts(j, QC)]
                                .rearrange("t p q -> p t q"),
                            )
                            nc.sync.dma_start(
                                bt[:, 2:4, 256:],
                                expb[h, 4 * tt + 2:4 * tt + 4, :,
                                     512 * j + 256:512 * j + 512]
                                .rearrange("t p q -> p t q"),
                            )
                        else:
                            bt_dma = nc.sync.dma_start(
                                bt[:],
                                expb[h, 4 * tt:4 * tt + 4, :, ts(j, QC)]
                                .rearrange("t p q -> p t q"),
                            )
                        if n_bt_dmas[0] < 4:
                            # don't let early expb prefetch steal HBM
                            # bandwidth from the critical-path xT load
                            add_dep_helper(
                                bt_dma.ins, xT_dmas[DC - 1].ins,
                                reason="expb prefetch after first-half xT",
                            )
                        n_bt_dmas[0] += 1
                        for pr in range(2):  # two k-tile pairs per tt-group
                            # diagonal narrowing: in the last tt-group the
                            # second pair's tiles (k >= 512j+256) only see
                            # q >= 512j+256, so compute the upper half-chunk
                            # only (the skipped region is causally masked ->
                            # contributes exactly 0)
                            q0 = 256 if (tt == j and pr == 1) else 0
                            pss = ps.tile([128, 2, QC], F32, tag="ps")
                            for half in range(2):
                                t = 4 * tt + 2 * pr + half
                                k_sl, q_sl = qk_slices(h, t, j)
                                nc.tensor.matmul(
                                    pss[:, half, q0:], k_sl, q_sl[:, q0:],
                                    start=True, stop=True,
                                )
                            es = expsp.tile([128, 2, QC], F16, tag="es")
                            nc.scalar.activation(
                                es[:, :, q0:], pss[:, :, q0:], EXP
                            )
                            et = expp.tile([128, 2, QC], F16, tag="et")
                            nc.vector.tensor_mul(
                                et[:, :, q0:], es[:, :, q0:],
                                bt[:, 2 * pr:2 * pr + 2, q0:],
                            )
                            for half in range(2):
                                t = 4 * tt + 2 * pr + half
                                nc.tensor.matmul(
                                    po[:, q0:], v1[:, t, h, :],
                                    et[:, half, q0:],
                                    start=(t == 0), stop=(t == nkt - 1),
                                )
                            if projq and slot % stride == 0:
                                projq.pop(0)()
                            slot += 1
                    normq.append(make_norm(po, h, j))
                for i0 in range(4 * j, 4 * j + 4, 2):
                    normq.append(make_wo(i0, tail=(j == NQC - 1)))
            for fn in normq + projq:
                fn()

    nc.compile()
    return nc


def get_program():
    if "nc" not in _prog_cache:
        _prog_cache["nc"] = _build_program()
    return _prog_cache["nc"]


def make_in_maps(hidden_states, attention_mask, position_bias, Wq, Wk, Wv, Wo):
    hs = np.ascontiguousarray(np.asarray(hidden_states, dtype=np.float32))
    am = np.asarray(attention_mask, dtype=np.float32)
    pb = np.asarray(position_bias, dtype=np.float32)
    wq = np.asarray(Wq, dtype=np.float32) * np.float32(1.0 / np.sqrt(DKV))
    wk = np.asarray(Wk, dtype=np.float32)
    wv_ = np.asarray(Wv, dtype=np.float32)
    wo_ = np.asarray(Wo, dtype=np.float32)

    # causal addend in [k, q] indexing: NEG where k > q
    kk = np.arange(Q, dtype=np.int64)
    causal_T = np.where(kk[:, None] > kk[None, :], np.float32(NEG), np.float32(0.0))
    causal_T = causal_T.astype(np.float32)

    in_maps = []
    for core in range(NCORES):
        b, g = divmod(core, NCORES // B)
        h0 = g * HPC
        # X^T chunked: [128, DC, Q], [p, c, q] = hs[b, q, 128c+p]
        xT = np.ascontiguousarray(
            hs[b].T.reshape(DC, 128, Q).transpose(1, 0, 2)
        ).astype(np.float16)
        # wqk: [128, DC, 384]: [Wq'01 | Wk01 | Wq'2 | Wk2]
        wq_sl = wq[:, h0 * DKV:(h0 + HPC) * DKV]
        wk_sl = wk[:, h0 * DKV:(h0 + HPC) * DKV]
        wqk = np.concatenate([
            wq_sl[:, 0:128], wk_sl[:, 0:128],
            wq_sl[:, 128:192], wk_sl[:, 128:192],
        ], axis=1)  # (D, 384)
        wqk = np.ascontiguousarray(
            wqk.reshape(DC, 128, 2 * HPC * DKV).transpose(1, 0, 2)
        ).astype(np.float16)
        # wv: [128, DC, HPC*DKV]
        wv_sl = wv_[:, (h0) * DKV:(h0 + HPC) * DKV].reshape(DC, 128, HPC * DKV)
        wv_sl = np.ascontiguousarray(wv_sl.transpose(1, 0, 2)).astype(np.float16)
        # wo: [DKV, HPC, D]: [p, h, n] = Wo[(h0+h)*DKV + p, n]
        wo_sl = np.ascontiguousarray(
            wo_[h0 * DKV:(h0 + HPC) * DKV, :]
        ).astype(np.float16)
        # expb: [HPC, NKT, 128, Q]: exp(biasT + causal + mask_k); masked -> 0
        # (attention_mask indexes k, which is the row dim of the transposed
        # bias, so it folds in as a per-row addend before the exp)
        bT = pb[0, h0:h0 + HPC].transpose(0, 2, 1) + causal_T[None]
        bT += am[b, 0, 0][None, :, None]
        bT = np.exp(bT, out=bT)
        bT = np.ascontiguousarray(bT.reshape(HPC, NKT, 128, Q)).astype(np.float16)
        in_maps.append({
            "xT": xT, "wqk": wqk, "wv": wv_sl, "wo": wo_sl,
            "expb": bT,
        })
    return in_maps


def kernel(hidden_states, attention_mask, position_bias, Wq, Wk, Wv, Wo):
    from concourse.bass_utils import run_bass_kernel_spmd

    nc = get_program()
    in_maps = make_in_maps(
        hidden_states, attention_mask, position_bias, Wq, Wk, Wv, Wo
    )
    res = run_bass_kernel_spmd(nc, in_maps, list(range(NCORES)))
    out = np.zeros((B, Q, D), dtype=np.float32)
    for core in range(NCORES):
        b = core // (NCORES // B)
        out[b] += res.results[core]["out"].reshape(Q, D).astype(np.float32)
    return out


# revision 86
# speedup vs baseline: 1.1738x; 1.0145x over previous
"""Bass/Tile TRN2 kernel for nn_MultiHeadAttention_4329327034628.

Multi-head self-attention with additive position bias + causal mask
(T5-style), B=2, Q=2048, D=768, H=12, DKV=64, fp32.

Sharding over 8 NeuronCores: core k -> (batch b = k//4, head-group
g = k%4 of 3 heads).  Each core computes its heads' attention and a
partial output projection (attn @ Wo_slice); the host sums the 4
partials per batch (the post-Wo all-reduce done at gather time).

Device-side layout strategy (no on-chip transposes needed):
  - host ships X^T (D on partitions) -> QKV projections contract D.
  - Q^T, K^T kept as [dkv, q]; scores computed transposed:
      scores^T [k, q] = lhsT(K^T slice).T @ rhs(Q^T)   (contract dkv)
  - position_bias is pre-transposed on host to [k, q] tiles, the causal
    NEG added, and *exponentiated* (expb = exp(biasT + causal), fp16):
    exp(s + b) = exp(s) * exp(b), so the device does ACT exp(s) followed
    by a cheap fp16 2x-mode DVE multiply -- no fp32 PSUM add needed.
    Masked entries have expb == 0 exactly -> probs match the reference.
  - attention_mask indexes k = partitions -> fused into the Exp
    activation as a per-partition bias.
  - softmax without max-subtraction (scores bounded by ~ +-10).
  - row-sum of exp fused into the AV matmul via a ones column:
      lhsT = [V_h | 1] [128k, 65] -> out rows 0..63 = out^T, row 64 = sum.
  - normalization: recip(sum) broadcast via ones-matmul, DVE multiply.
  - Wo: lhsT = attnT_h [64, 128q], rhs = Wo slice [64, 384] -> natural
    [q, D] partial output, DMA'd out.
"""

import numpy as np

B, Q, D, H, DKV = 2, 2048, 768, 12, 64
HPC = 3              # heads per core
NCORES = 8
NEG = -30000.0       # causal mask addend; exp(x + NEG) == 0.0 in fp32
QC = 512             # q chunk (moving dim)
KT = 128             # k tile (partition dim)
NQC = Q // QC        # 4
NKT = Q // KT        # 16
DC = D // 128        # 6 contraction chunks

_prog_cache = {}


def _build_program():
    import concourse.bass as bass
    import concourse.tile as tile
    from concourse import bacc, mybir
    from concourse.bass import ts

    F32 = mybir.dt.float32
    F16 = mybir.dt.float16
    EXP = mybir.ActivationFunctionType.Exp

    nc = bacc.Bacc("TRN2", target_bir_lowering=False, debug=False)

    xT = nc.dram_tensor("xT", [128, DC, Q], F16, kind="ExternalInput").ap()
    # cols 0:128 = Wq' heads {0,1}; 128:256 = Wk heads {0,1};
    # 256:320 = Wq' head 2; 320:384 = Wk head 2
    wqk = nc.dram_tensor("wqk", [128, DC, 2 * HPC * DKV], F16, kind="ExternalInput").ap()
    wv = nc.dram_tensor("wv", [128, DC, HPC * DKV], F16, kind="ExternalInput").ap()
    wo = nc.dram_tensor("wo", [HPC * DKV, D], F16, kind="ExternalInput").ap()
    expb = nc.dram_tensor("expb", [HPC, NKT, 128, Q], F16, kind="ExternalInput").ap()
    out = nc.dram_tensor("out", [NKT, 128, D], F16, kind="ExternalOutput").ap()

    with tile.TileContext(nc) as tc:
        with (
            nc.allow_low_precision(reason="fp16 matmul operands; fp32 psum accum"),
            tc.tile_pool(name="const", bufs=1) as const,
            tc.tile_pool(name="ps", bufs=3, space="PSUM") as ps,
            tc.tile_pool(name="projps", bufs=1, space="PSUM") as projps,
            tc.tile_pool(name="psO", bufs=1, space="PSUM") as psO,

            tc.tile_pool(name="biasp", bufs=5) as biasp,
            tc.tile_pool(name="expsp", bufs=6) as expsp,
            tc.tile_pool(name="expp", bufs=6) as expp,
            tc.tile_pool(name="small", bufs=3) as small,
            tc.tile_pool(name="outp", bufs=3) as outp,
        ):
            # ---- stage A: load everything ----
            from concourse.tile import add_dep_helper
            wqk_sb = const.tile([128, DC, 2 * HPC * DKV], F16, tag="wqk")
            nc.sync.dma_start(wqk_sb[:], wqk[:])
            wv_sb = const.tile([128, DC, HPC * DKV], F16, tag="wv")
            nc.sync.dma_start(wv_sb[:], wv[:])
            # Wo stacked: [0:128] = heads {0,1} vertically, wo2 = head 2
            wo01_sb = const.tile([2 * DKV, D], F16, tag="wo01")
            nc.sync.dma_start(wo01_sb[:], wo[0:2 * DKV, :])
            wo2_sb = const.tile([DKV, D], F16, tag="wo2")
            nc.sync.dma_start(wo2_sb[:], wo[2 * DKV:, :])
            xT_sb = const.tile([128, DC, Q], F16, tag="xT")
            # first-half q-columns land first: chunk-0/1 projections only
            # read those, so the PE unblocks ~4us sooner
            xT_dmas = [
                nc.sync.dma_start(
                    xT_sb[:, c, ts(hf, Q // 2)], xT[:, c, ts(hf, Q // 2)]
                )
                for hf in range(2)
                for c in range(DC)
            ]
            ones1 = const.tile([1, DKV], F16, tag="ones1")
            nc.gpsimd.memset(ones1[:], 1.0)

            # ---- stage B: projections ----
            # Q^T / K^T storage. Heads 0,1 paired in [128, Q] tiles (head 0 =
            # rows 0:64, head 1 = rows 64:128, so scores-matmul operands share
            # a base partition); head 2 in separate [64, Q] tiles (base 0).
            qT01 = const.tile([128, Q], F16, tag="qT01")
            kT01 = const.tile([128, Q], F16, tag="kT01")
            # head 2: one M=128 group -> qkT2 rows 0:64 = Q^T, 64:128 = K^T;
            # K^T is then DMA-shifted down to kT2b rows 0:64 so the scores
            # matmul operands share base partition 0
            qkT2 = const.tile([128, Q], F16, tag="qkT2")
            kT2b = const.tile([DKV, Q], F16, tag="kT2b")
            # (lhsT weight slice, dest tile) per projection matmul group
            proj_groups = [
                ((0, 128), qT01), ((128, 256), kT01), ((256, 384), qkT2),
            ]
            def make_qk_unit(j, g, pool_tag=None):
                (w0, w1), dst = proj_groups[g]

                def qk_unit():
                    pool, tag = pool_tag or (projps, "pp")
                    p = pool.tile([w1 - w0, QC], F32, tag=tag, name="p")
                    for c in range(DC):
                        nc.tensor.matmul(
                            p[:], wqk_sb[:, c, w0:w1], xT_sb[:, c, ts(j, QC)],
                            start=(c == 0), stop=(c == DC - 1),
                        )
                    nc.scalar.copy(dst[:, ts(j, QC)], p[:])
                    if dst is qkT2:
                        nc.sync.dma_start(
                            kT2b[:, ts(j, QC)], qkT2[DKV:128, ts(j, QC)]
                        )
                return qk_unit

            def qk_slices(h, t, j):
                """(lhsT k-slice, rhs q-slice) for head h, k-tile t, q-chunk j."""
                if h == 0:
                    return kT01[0:DKV, ts(t, KT)], qT01[0:DKV, ts(j, QC)]
                if h == 1:
                    return kT01[DKV:128, ts(t, KT)], qT01[DKV:128, ts(j, QC)]
                return kT2b[:, ts(t, KT)], qkT2[0:DKV, ts(j, QC)]

            # V (natural [k, d]) with a ones column per head: [128, NKT, 3, 65]
            # per (tile t, head h): v1[:, t, h, 0:64] = V_h, v1[:, t, h, 64] = 1
            v1 = const.tile([128, NKT, HPC, DKV + 1], F16, tag="v1")
            nc.gpsimd.memset(v1[:], 1.0)

            def make_v_unit(t, pool_tag=None):
                def v_unit():
                    pool, tag = pool_tag or (projps, "pp")
                    pv = pool.tile([128, HPC * DKV], F32, tag=tag, name="pv")
                    for c in range(DC):
                        nc.tensor.matmul(
                            pv[:], xT_sb[:, c, ts(t, KT)], wv_sb[:, c, :],
                            start=(c == 0), stop=(c == DC - 1),
                        )
                    # single strided copy: [128, 3, 64] <- [128, (3 64)]
                    nc.vector.tensor_copy(
                        v1[:, t, :, 0:DKV],
                        pv[:].rearrange("p (h d) -> p h d", h=HPC),
                    )
                return v_unit

            # ---- stage C: attention (scores^T layout), stage D: Wo ----
            attnT01 = const.tile([2 * DKV, Q], F16, tag="attnT01")
            attnT2 = const.tile([DKV, Q], F16, tag="attnT2")
            def make_norm(po, h, j):
                # normalize: attnT_h[:, jq] = po[0:64] * (1/po[64]) bcast
                def norm():
                    rc = small.tile([1, QC], F16, tag="rc", name="rc")
                    nc.vector.reciprocal(rc[:], po[64:65, :])
                    bc = small.tile([DKV, QC], F16, tag="bc", name="bc")
                    nc.gpsimd.partition_broadcast(bc[:], rc[:])
                    if h == 0:
                        dst = attnT01[0:DKV, ts(j, QC)]
                    elif h == 2:
                        dst = attnT2[:, ts(j, QC)]
                    else:
                        dst = None
                    if dst is not None:
                        nc.vector.tensor_mul(dst, po[0:64, :], bc[:])
                    else:
                        # head 1 lands at partitions 64:128 of the stacked
                        # pair tile; DVE can't shift partitions, so stage at
                        # base 0 and DMA-shift (SBUF->SBUF moves are
                        # partition-agnostic)
                        stg = small.tile([DKV, QC], F16, tag="stg", name="stg")
                        nc.vector.tensor_mul(stg[:], po[0:64, :], bc[:])
                        nc.sync.dma_start(attnT01[DKV:128, ts(j, QC)], stg[:])
                return norm

            def make_wo(i0, tail=False):
                # Wo partial for q-tiles i0, i0+1 (2x128 q rows, one out DMA)
                def wo_i():
                    ot = outp.tile([128, 2, D], F16, tag="ot", name="ot")
                    for ii in range(2):
                        i = i0 + ii
                        for half in range(2):
                            if tail:
                                pool, tag = (ps, "ps") if half else (projps, "pp")
                                pf = pool.tile([128, 384], F32, tag=tag, name="pf")
                            else:
                                pf = projps.tile([128, 384], F32, tag="pp", name="pf")
                            hs = slice(384 * half, 384 * half + 384)
                            nc.tensor.matmul(
                                pf[:], attnT01[:, ts(i, KT)], wo01_sb[:, hs],
                                start=True, stop=False,
                            )
                            nc.tensor.matmul(
                                pf[:], attnT2[:, ts(i, KT)], wo2_sb[:, hs],
                                start=False, stop=True,
                            )
                            if half == 0:
                                nc.vector.tensor_copy(ot[:, ii, 0:384], pf[:])
                            elif i < 8:
                                # early chunks: ACT has slack; late chunks
                                # are ACT-bound, keep copies off its queue
                                nc.scalar.copy(ot[:, ii, 384:768], pf[:])
                            else:
                                nc.vector.tensor_copy(ot[:, ii, 384:768], pf[:])
                    nc.sync.dma_start(
                        out[i0:i0 + 2].rearrange("i p n -> p i n"), ot[:]
                    )
                return wo_i

            # Software pipelining via two drip queues:
            #  - normq: deferred normalization + Wo closures (FIFO keeps
            #    norm(h,j) ahead of wo(j,*) which reads normalized attnT);
            #    popped at (h,j) / tt boundaries so po slots recycle.
            #  - projq: projection units for q-chunk j+1 (Q/K chunk matmuls,
            #    V k-tiles), dripped one per t-step of attention(j) so the
            #    serial projection phase disappears into attention's PE gaps.
            normq = []
            projq = []
            n_bt_dmas = [0]
            # chunk-0 projections must precede attention(0); attention is not
            # running yet, so spread them over the idle pair-pool psum slots
            # to pipeline instead of serializing on the single "pp" slot
            startup_slots = [(projps, "pp"), (ps, "ps"), (psO, "po")]
            for g in range(len(proj_groups)):
                make_qk_unit(0, g, startup_slots[g % 3])()
            for t in range(4):
                make_v_unit(t, startup_slots[t % 3])()

            for j in range(NQC):
                # any leftover proj(j) units must be emitted before
                # attention(j) reads their outputs
                while projq:
                    projq.pop(0)()
                if j + 1 < NQC:
                    for g in range(len(proj_groups)):
                        projq.append(make_qk_unit(j + 1, g))
                    for t in range(4 * (j + 1), 4 * (j + 1) + 4):
                        projq.append(make_v_unit(t))
                # drip projections evenly: 8 units over this j's 12(j+1)
                # t-steps, front-loaded enough to finish before attn(j+1)
                stride = max(1, (6 * (j + 1)) // 9)
                slot = 0
                for h in range(HPC):
                    # free po slots before claiming one (po bufs=2)
                    for _ in range(min(2, len(normq))):
                        normq.pop(0)()
                    po = psO.tile([65, QC], F32, tag="po")
                    nkt = 4 * j + 4  # causal: k-tiles 0..4j+3
                    for tt in range(j + 1):  # batched expb DMA: 4 k-tiles
                        if normq:
                            normq.pop(0)()
                        bt = biasp.tile([128, 4, QC], F16, tag="bt")
                        if tt == j:
                            # diagonal group: second pair only needs the
                            # upper half of the q-chunk
                            bt_dma = nc.sync.dma_start(
                                bt[:, 0:2, :],
                                expb[h, 4 * tt:4 * tt + 2, :, ts(j, QC)]
                                .rearrange("t p q -> p t q"),
                            )
                            nc.sync.dma_start(
                                bt[:, 2:4, 256:],
                                expb[h, 4 * tt + 2:4 * tt + 4, :,
                                     512 * j + 256:512 * j + 512]
                                .rearrange("t p q -> p t q"),
                            )
                        else:
                            bt_dma = nc.sync.dma_start(
                                bt[:],
                                expb[h, 4 * tt:4 * tt + 4, :, ts(j, QC)]
                                .rearrange("t p q -> p t q"),
                            )
                        if n_bt_dmas[0] < 4:
                            # don't let early expb prefetch steal HBM
                            # bandwidth from the critical-path xT load
                            add_dep_helper(
                                bt_dma.ins, xT_dmas[DC - 1].ins,
                                reason="expb prefetch after first-half xT",
                            )
                        n_bt_dmas[0] += 1
                        for pr in range(2):  # two k-tile pairs per tt-group
                            # diagonal narrowing: in the last tt-group the
                            # second pair's tiles (k >= 512j+256) only see
                            # q >= 512j+256, so compute the upper half-chunk
                            # only (the skipped region is causally masked ->
                            # contributes exactly 0)
                            q0 = 256 if (tt == j and pr == 1) else 0
                            pss = ps.tile([128, 2, QC], F32, tag="ps")
                            for half in range(2):
                                t = 4 * tt + 2 * pr + half
                                k_sl, q_sl = qk_slices(h, t, j)
                                nc.tensor.matmul(
                                    pss[:, half, q0:], k_sl, q_sl[:, q0:],
                                    start=True, stop=True,
                                )
                            es = expsp.tile([128, 2, QC], F16, tag="es")
                            nc.scalar.activation(
                                es[:, :, q0:], pss[:, :, q0:], EXP
                            )
                            et = expp.tile([128, 2, QC], F16, tag="et")
                            nc.vector.tensor_mul(
                                et[:, :, q0:], es[:, :, q0:],
                                bt[:, 2 * pr:2 * pr + 2, q0:],
                            )
                            for half in range(2):
                                t = 4 * tt + 2 * pr + half
                                nc.tensor.matmul(
                                    po[:, q0:], v1[:, t, h, :],
                                    et[:, half, q0:],
                                    start=(t == 0), stop=(t == nkt - 1),
                                )
                            if projq and slot % stride == 0:
                                projq.pop(0)()
                            slot += 1
                    make_norm(po, h, j)()
                for i0 in range(4 * j, 4 * j + 4, 2):
                    normq.append(make_wo(i0, tail=(j == NQC - 1)))
            for fn in normq + projq:
                fn()

    nc.compile()
    return nc


def get_program():
    if "nc" not in _prog_cache:
        _prog_cache["nc"] = _build_program()
    return _prog_cache["nc"]


def make_in_maps(hidden_states, attention_mask, position_bias, Wq, Wk, Wv, Wo):
    hs = np.ascontiguousarray(np.asarray(hidden_states, dtype=np.float32))
    am = np.asarray(attention_mask, dtype=np.float32)
    pb = np.asarray(position_bias, dtype=np.float32)
    wq = np.asarray(Wq, dtype=np.float32) * np.float32(1.0 / np.sqrt(DKV))
    wk = np.asarray(Wk, dtype=np.float32)
    wv_ = np.asarray(Wv, dtype=np.float32)
    wo_ = np.asarray(Wo, dtype=np.float32)

    # causal addend in [k, q] indexing: NEG where k > q
    kk = np.arange(Q, dtype=np.int64)
    causal_T = np.where(kk[:, None] > kk[None, :], np.float32(NEG), np.float32(0.0))
    causal_T = causal_T.astype(np.float32)

    in_maps = []
    for core in range(NCORES):
        b, g = divmod(core, NCORES // B)
        h0 = g * HPC
        # X^T chunked: [128, DC, Q], [p, c, q] = hs[b, q, 128c+p]
        xT = np.ascontiguousarray(
            hs[b].T.reshape(DC, 128, Q).transpose(1, 0, 2)
        ).astype(np.float16)
        # wqk: [128, DC, 384]: [Wq'01 | Wk01 | Wq'2 | Wk2]
        wq_sl = wq[:, h0 * DKV:(h0 + HPC) * DKV]
        wk_sl = wk[:, h0 * DKV:(h0 + HPC) * DKV]
        wqk = np.concatenate([
            wq_sl[:, 0:128], wk_sl[:, 0:128],
            wq_sl[:, 128:192], wk_sl[:, 128:192],
        ], axis=1)  # (D, 384)
        wqk = np.ascontiguousarray(
            wqk.reshape(DC, 128, 2 * HPC * DKV).transpose(1, 0, 2)
        ).astype(np.float16)
        # wv: [128, DC, HPC*DKV]
        wv_sl = wv_[:, (h0) * DKV:(h0 + HPC) * DKV].reshape(DC, 128, HPC * DKV)
        wv_sl = np.ascontiguousarray(wv_sl.transpose(1, 0, 2)).astype(np.float16)
        # wo: [DKV, HPC, D]: [p, h, n] = Wo[(h0+h)*DKV + p, n]
        wo_sl = np.ascontiguousarray(
            wo_[h0 * DKV:(h0 + HPC) * DKV, :]
        ).astype(np.float16)
        # expb: [HPC, NKT, 128, Q]: exp(biasT + causal + mask_k); masked -> 0
        # (attention_mask indexes k, which is the row dim of the transposed
        # bias, so it folds in as a per-row addend before the exp)
        bT = pb[0, h0:h0 + HPC].transpose(0, 2, 1) + causal_T[None]
        bT += am[b, 0, 0][None, :, None]
        bT = np.exp(bT, out=bT)
        bT = np.ascontiguousarray(bT.reshape(HPC, NKT, 128, Q)).astype(np.float16)
        in_maps.append({
            "xT": xT, "wqk": wqk, "wv": wv_sl, "wo": wo_sl,
            "expb": bT,
        })
    return in_maps


def kernel(hidden_states, attention_mask, position_bias, Wq, Wk, Wv, Wo):
    from concourse.bass_utils import run_bass_kernel_spmd

    nc = get_program()
    in_maps = make_in_maps(
        hidden_states, attention_mask, position_bias, Wq, Wk, Wv, Wo
    )
    res = run_bass_kernel_spmd(nc, in_maps, list(range(NCORES)))
    out = np.zeros((B, Q, D), dtype=np.float32)
    for core in range(NCORES):
        b = core // (NCORES // B)
        out[b] += res.results[core]["out"].reshape(Q, D).astype(np.float32)
    return out
